# revision 1
# baseline (speedup 1.0000x reference)
"""Trainium2 Bass kernel for nn_BayerFeatureExtractor.

Input:  bayer [4, 1, 768, 768] f32.  Output: [4, 30, 768, 768] f32.

Sharding: pure data-parallel over 8 cores: core i handles batch b = i//2,
row-half h = i%2 (output rows [h*384, (h+1)*384)).

Per-core kernel: every convolution is expressed as a small set of banded
matmuls on the TensorEngine (contraction over image rows = SBUF partitions,
float32r at N>=256 streams 1 column/cycle), with Bayer-phase weights folded
into the bands / column-parity interleaves.  Horizontal finishing taps and
elementwise chains run on the Vector/Scalar engines.  Each core processes
4 row-tiles (96 output rows) x 2 col-blocks (384 output cols + halo).

Geometry (validated bit-exact vs reference by a numpy prototype):
  BT = bayer_pad[r0:r0+104, c0:c0+392]; BT partition k <-> out row r0+k-4
  ext maps: M=100, ext row x <-> out row r0-2+x (reads BT partitions x+2+dy)
  stage-2: M=96, out row y reads ext partitions y+2+dy
  stage-1 pure-V psum N=392 (cols c0-4..); direct/ext N=388 (cols c0-2..);
  stage-2 psum N=384 (central).
"""
import math
import os
import sys
from contextlib import ExitStack

import numpy as np

for _p in ('/opt/trn_rl_repo', '/root/.axon_site/_ro/trn_rl_repo'):
    if os.path.isdir(_p) and _p not in sys.path:
        sys.path.insert(0, _p)

import concourse.bass as bass
import concourse.bacc as bacc
import concourse.mybir as mybir
import concourse.tile as tile
from concourse.bass_utils import run_bass_kernel_spmd

F32 = mybir.dt.float32
F32R = mybir.dt.float32r
BF16 = mybir.dt.bfloat16
AL = mybir.AluOpType
AF = mybir.ActivationFunctionType

EPS = 1e-6
K1, M1 = 104, 100
K2, M2 = 100, 96

# ----------------------------------------------------------------------------
# constants (identical math to reference._build_kernels)
# ----------------------------------------------------------------------------


def _norm(k):
    k = k - k.mean()
    return (k / max(float(np.abs(k).sum()), 1e-6)).astype(np.float32)


def _gabor(theta, size=5, sigma=1.1, wavelength=3.0, gamma=0.65):
    r = size // 2
    c = np.arange(-r, r + 1, dtype=np.float32)
    yy, xx = np.meshgrid(c, c, indexing='ij')
    xt = xx * math.cos(theta) + yy * math.sin(theta)
    yt = -xx * math.sin(theta) + yy * math.cos(theta)
    env = np.exp(-(xt ** 2 + (gamma * yt) ** 2) / (2.0 * sigma * sigma))
    return _norm(env * np.cos(2.0 * math.pi * xt / wavelength))


def _dct(size=5, u=2, v=2):
    c = np.arange(size, dtype=np.float32)
    return _norm(np.outer(np.cos(math.pi * (c + 0.5) * v / size),
                          np.cos(math.pi * (c + 0.5) * u / size)))


def build_kernels():
    f32 = np.float32
    k3 = np.stack([
        _norm(np.array([[-1, 0, 1], [-2, 0, 2], [-1, 0, 1]], f32)),
        _norm(np.array([[-1, -2, -1], [0, 0, 0], [1, 2, 1]], f32)),
        _norm(np.array([[-2, -1, 0], [-1, 0, 1], [0, 1, 2]], f32)),
        _norm(np.array([[0, 1, 2], [-1, 0, 1], [-2, -1, 0]], f32)),
        np.array([[0, 1, 0], [1, -4, 1], [0, 1, 0]], f32),
        np.array([[0, 0, 0], [1, -2, 1], [0, 0, 0]], f32),
        np.array([[0, 1, 0], [0, -2, 0], [0, 1, 0]], f32),
        np.array([[1, 0, -1], [0, 0, 0], [-1, 0, 1]], f32) / 4.0,
        np.array([[0, .25, 0], [.25, 0, .25], [0, .25, 0]], f32),
        _norm(np.array([[1, -2, 1], [-2, 4, -2], [1, -2, 1]], f32)),
    ])
    ii, jj = np.indices((5, 5))
    s = np.sin(2.0 * math.pi * np.arange(5, dtype=f32) / 5.0)
    c = np.cos(2.0 * math.pi * np.arange(5, dtype=f32) / 5.0)
    k5 = np.stack([
        _norm(((-1.0) ** (ii + jj)).astype(f32)),   # cb
        _norm(((-1.0) ** jj).astype(f32)),          # sh
        _norm(((-1.0) ** ii).astype(f32)),          # sv
        _norm(np.tile(s, (5, 1))),                  # sinx
        _norm(np.tile(s.reshape(5, 1), (1, 5))),    # siny
        _norm(np.tile(c, (5, 1))),                  # phx
        _norm(np.tile(c.reshape(5, 1), (1, 5))),    # phy
        _gabor(math.pi / 4.0),                      # g45
        _gabor(3.0 * math.pi / 4.0),                # g135
        _dct(),                                     # dct
    ])
    ha = np.array([-0.25, 0.5, 0.5, 0.5, -0.25], f32)
    return k3, k5, ha


def banded1(col5, off=2, M=M1):
    B = np.zeros((K1, M), np.float32)
    for x in range(M):
        for dy in range(-2, 3):
            k = x + off + dy
            if 0 <= k < K1:
                B[k, x] = col5[dy + 2]
    return B


def banded2(col5, scale=1.0):
    B = np.zeros((K2, M2), np.float32)
    for y in range(M2):
        for dy in range(-2, 3):
            k = y + 2 + dy
            if 0 <= k < K2:
                B[k, y] = col5[dy + 2] * scale
    return B


def pad5(col3):
    z = np.zeros(5, np.float32)
    z[1:4] = np.asarray(col3, np.float32)
    return z


def build_stage1_lhs():
    """Stage-1 lhsT matrices.  Two kinds:
    - ext-V (off=2, M=100): V3a, V3b, Vodd, Veven  (psum N=392)
    - central direct (off=4, M=96): everything consumed only centrally
      (psum N=384, rhs col offset 4+dx)
    All matrices packed at column stride M1=100 (central ones zero-padded).
    """
    k3, k5, ha = build_kernels()
    t5 = np.array([1, 2, 3, 2, 1], np.float32) / 9.0
    mats, idx = [], {}

    def add(name, mlist, dxs=None, M=96):
        idx[name] = (len(mats), len(mlist))
        idx[name + '_M'] = M
        if dxs is not None:
            idx[name + '_dx'] = dxs
        mats.extend(mlist)

    add('V3a', [banded1(pad5([1, 2, 1]))], M=100)
    add('V3b', [banded1(pad5([-1, 0, 1]))], M=100)
    Bo = banded1(t5)
    Be = Bo.copy()
    kk = np.arange(K1) % 2
    Bo = Bo * kk[:, None]
    Be = Be * (1 - kk)[:, None]
    add('Vodd', [Bo.astype(np.float32)], M=100)
    add('Veven', [Be.astype(np.float32)], M=100)

    def direct(name, ker):
        kh, kw = ker.shape
        r = kw // 2
        ms, dxs = [], []
        for dx in range(-r, r + 1):
            col = ker[:, dx + r]
            if not np.any(col != 0):
                continue
            c5 = pad5(col) if kh == 3 else col.astype(np.float32)
            ms.append(banded1(c5, off=4, M=96))
            dxs.append(dx)
        add(name, ms, dxs, M=96)

    direct('gdm', k3[2])
    direct('gda', k3[3])
    direct('sumd', k3[0] + k3[1] + k3[2] + k3[3])
    direct('lap', k3[4])
    direct('hd', k3[5] - k3[6])
    direct('hxy', k3[7])
    direct('gcross', k3[8])
    direct('hf', k3[9])
    direct('gxC', k3[0])
    direct('gyC', k3[1])
    for nm, kk5 in zip(['cb', 'sh', 'sv', 'sinx', 'siny', 'phx', 'phy',
                        'g45', 'g135', 'dct'], k5):
        direct(nm, kk5)
    IshC = banded1(pad5([0, 1, 0]), off=4, M=96)
    add('gh', [IshC * ha[dx + 2] for dx in range(-2, 3)],
        [-2, -1, 0, 1, 2], M=96)
    add('gv', [banded1(ha, off=4, M=96)], M=96)
    add('bayC', [IshC], M=96)

    packed = np.zeros((len(mats), K1, M1), np.float32)
    for i, m in enumerate(mats):
        packed[i, :, :m.shape[1]] = m
    return packed, idx


def build_stage2_lhs():
    t5 = np.array([1, 2, 3, 2, 1], np.float32) / 9.0
    ones5 = np.ones(5, np.float32) / 5.0
    I = np.eye(K2, dtype=np.float32)
    a_mats = np.stack([I * t5[dx + 2] for dx in range(-2, 3)])  # [5,100,100]
    j_mats = np.stack([banded2(t5, t5[dx + 2]) for dx in range(-2, 3)])
    box_mat = banded2(ones5, 1.0 / 5.0)
    return a_mats, j_mats, box_mat


PAT_NAMES = ['IVR', 'IVB', 'IVG', 'IVGR', 'IVGB', 'GM', 'RM', 'BM']


def build_patterns():
    t5 = np.array([1, 2, 3, 2, 1], np.float32) / 9.0

    def mfun(ch, rp, cp):
        return {
            'r': float(rp == 1 and cp == 0),
            'b': float(rp == 0 and cp == 1),
            'gr': float(rp == 1 and cp == 1),
            'gb': float(rp == 0 and cp == 0),
            'g': float((rp == 1 and cp == 1) or (rp == 0 and cp == 0)),
        }[ch]

    P, W = 128, 776
    pp = np.arange(P)[:, None] % 2
    cc = np.arange(W)[None, :] % 2  # abs col parity == m%2 (offset -4 even)
    pats = []
    for ch in ['r', 'b', 'g', 'gr', 'gb']:
        v = np.zeros((2, 2), np.float32)
        for rp in range(2):
            for cp in range(2):
                d = sum(t5[dy + 2] * t5[dx + 2]
                        * mfun(ch, (rp + dy) % 2, (cp + dx) % 2)
                        for dy in range(-2, 3) for dx in range(-2, 3))
                v[rp, cp] = 1.0 / max(d, EPS)
        pats.append(v[pp, cc].astype(np.float32))
    masks = []
    for ch in ['r', 'g', 'b']:
        v = np.array([[mfun(ch, rp, cp) for cp in range(2)]
                      for rp in range(2)], np.float32)
        masks.append(v[pp, cc].astype(np.float32))
    return np.stack(pats), np.stack(masks)  # [5,128,776], [3,128,776] (r,g,b)


def build_rowsign(h):
    sg = np.ones((4, M1), np.float32)
    for t in range(4):
        for x in range(M1):
            r = h * 384 + 96 * t - 2 + x
            if r < 0 or r >= 768:
                sg[t, x] = -1.0
    return sg.T.copy()  # [100, 4]


# staging channel layout: two contiguous output ranges (ext channels 0,1
# [gx,gy] and 15,16 [rg,bg] are DMA'd straight from their ext tiles)
STG_A = list(range(2, 15))    # gdm..gpd
STG_B = list(range(17, 30))   # gir..highband
CH = {n: i for i, n in enumerate([
    'gx', 'gy', 'gdm', 'gda', 'grad_mag', 'lap', 'lam_max', 'lam_min',
    'aniso', 'dir_var', 'orient_e', 'r_m', 'g_m', 'b_m', 'gpd', 'rg', 'bg',
    'gir', 'dgc', 'chroma_mag', 'cdv', 'cb_e', 'sh', 'sv', 'ax', 'ay',
    'phase_e', 'sx', 'sy', 'highband'])}


# ----------------------------------------------------------------------------
# kernel builder
# ----------------------------------------------------------------------------

def build_nc():
    lhs1_np, idx = build_stage1_lhs()
    a_mats, j_mats, box_mat = build_stage2_lhs()
    k3, _, _ = build_kernels()
    n1 = lhs1_np.shape[0]

    nc = bacc.Bacc(None, target_bir_lowering=False)
    bayer_d = nc.dram_tensor('bayer_pad', [392, 776], BF16, kind='ExternalInput')
    lhs1_d = nc.dram_tensor('lhs1', [K1, n1 * M1], BF16, kind='ExternalInput')
    lhs2_d = nc.dram_tensor('lhs2', [K2, 5 * 100 + 5 * 96 + 96], BF16,
                            kind='ExternalInput')
    pat_d = nc.dram_tensor('pats', [128, 5 * 776], F32, kind='ExternalInput')
    mask_d = nc.dram_tensor('masks', [128, 3 * 776], BF16,
                            kind='ExternalInput')
    rsg_d = nc.dram_tensor('rowsgn', [M1, 4], F32, kind='ExternalInput')
    out_d = nc.dram_tensor('out', [30, 384, 768], F32, kind='ExternalOutput')

    with tile.TileContext(nc) as tc, ExitStack() as ctx:
        cpool = ctx.enter_context(tc.tile_pool(name='const', bufs=1))
        inpool = ctx.enter_context(tc.tile_pool(name='inp', bufs=3))
        wpool = ctx.enter_context(tc.tile_pool(name='work', bufs=1))
        tpool = ctx.enter_context(tc.tile_pool(name='tmp', bufs=1))
        spool = ctx.enter_context(tc.tile_pool(name='stage', bufs=2))
        pspool = ctx.enter_context(
            tc.tile_pool(name='ps', bufs=8, space='PSUM'))

        epsT = cpool.tile([128, 1], F32, tag='epsT', name='epsT')
        eps4T = cpool.tile([128, 1], F32, tag='eps4T', name='eps4T')
        nc.vector.memset(epsT[:], EPS)
        nc.vector.memset(eps4T[:], 4.0 * EPS)
        lhs1_t = cpool.tile([K1, n1 * M1], BF16, tag='lhs1')
        lhs2_t = cpool.tile([K2, 1076], BF16, tag='lhs2')
        pat_t = cpool.tile([128, 5, 776], F32, tag='pats')
        mask_t = cpool.tile([128, 3, 776], BF16, tag='masks')
        rsg_t = cpool.tile([M1, 4], F32, tag='rsg')
        nc.sync.dma_start(lhs1_t[:], lhs1_d[:])
        nc.sync.dma_start(lhs2_t[:], lhs2_d[:])

        def l1(name, i=0):
            s, _ = idx[name]
            M = idx[name + '_M']
            return lhs1_t[:, (s + i) * M1:(s + i) * M1 + M]

        def l2A(i):
            return lhs2_t[:, i * 100:(i + 1) * 100]

        def l2J(i):
            return lhs2_t[:, 500 + i * 96:500 + (i + 1) * 96]

        l2box = lambda: lhs2_t[:, 980:1076]

        def MM(ps, lh, rh, start, stop):
            nc.tensor.matmul(ps, lh, rh, start=start, stop=stop)

        def stt(out, in0, w, in1):
            nc.vector.scalar_tensor_tensor(out, in0, float(w), in1,
                                           AL.mult, AL.add)

        def hconv(dst, taps, tmps):
            n = len(taps)
            if n == 1:
                nc.vector.tensor_scalar(dst, taps[0][0], float(taps[0][1]),
                                        None, AL.mult)
                return
            cur = tmps[0]
            nc.vector.tensor_scalar(cur, taps[0][0], float(taps[0][1]),
                                    None, AL.mult)
            for i in range(1, n - 1):
                nxt = tmps[i % 2]
                if nxt is cur:
                    nxt = tmps[(i + 1) % 2]
                stt(nxt, taps[i][0], taps[i][1], cur)
                cur = nxt
            stt(dst, taps[n - 1][0], taps[n - 1][1], cur)

        def act(out, in_, func, bias=0.0, scale=1.0):
            if isinstance(bias, float) and bias != 0.0:
                bt = eps4T if bias == 4.0 * EPS else epsT
                bias = bt[0:out.shape[0], :]
            nc.scalar.activation(out, in_, func, bias=bias, scale=scale)

        gx_w = [(dx, float(k3[0][1, dx + 1]) / 2.0) for dx in (-1, 1)]
        gy_w = [(dx, float(k3[1][2, dx + 1])) for dx in (-1, 0, 1)]
        t5 = np.array([1, 2, 3, 2, 1], np.float32) / 9.0

        for t in range(4):
            r0 = 96 * t
            for cbi in range(2):
                c0 = 384 * cbi
                BT = inpool.tile([K1, 392], BF16, tag='BT')
                nc.sync.dma_start(BT[:], bayer_d[r0:r0 + 104, c0:c0 + 392])
                if t == 0 and cbi == 0:
                    nc.sync.dma_start(
                        pat_t[:], pat_d[:].rearrange('p (n w) -> p n w', n=5))
                    nc.sync.dma_start(
                        mask_t[:], mask_d[:].rearrange('p (n w) -> p n w', n=3))
                    nc.sync.dma_start(rsg_t[:], rsg_d[:])

                def pv(pi, w0, wn, pn):
                    # pattern view: partitions [0:pn], master col c0+w0..c0+wn
                    return pat_t[0:pn, pi, c0 + w0:c0 + wn]

                stA = spool.tile([96, 11, 384], F32, tag='stA')
                stB = spool.tile([96, 10, 384], F32, tag='stB')
                STB_CH = [20, 21, 22, 23, 26, 27, 28, 29, 17, 18]

                def stg(name):
                    i = CH[name]
                    if i < 11:
                        return stA[:, i, :]
                    return stB[:, STB_CH.index(i), :]

                def ps_new(shape, tag='ps'):
                    return pspool.tile(shape, F32, tag=tag, name=tag)

                def tmp(tag, shape=(96, 384), dt=F32):
                    return tpool.tile(list(shape), dt, tag=tag, name=tag)

                def cser(name, pstile):
                    """central direct series: psum [96,384]"""
                    s, cnt = idx[name]
                    dxs = idx.get(name + '_dx', [0] * cnt)
                    for i in range(cnt):
                        off = 4 + dxs[i]
                        MM(pstile[:], l1(name, i), BT[:, off:off + 384],
                           i == 0, i == cnt - 1)

                def vser(name, pstile):
                    """ext-V series: psum [100,392]"""
                    MM(pstile[:], l1(name), BT[:, 0:392], True, True)

                hA = tmp('hA', (M1, 392))
                hB = tmp('hB', (M1, 392))

                # ============ fills chain
                vo_ps = ps_new([M1, 392])
                vser('Vodd', vo_ps)
                ve_ps = ps_new([M1, 392])
                vser('Veven', ve_ps)
                voddE = wpool.tile([M1, 392], BF16, tag='voddE')
                vevenE = wpool.tile([M1, 392], BF16, tag='vevenE')
                act(voddE[:], vo_ps[:], AF.Copy)
                act(vevenE[:], ve_ps[:], AF.Copy)

                AeO = ps_new([M1, 388])
                AoO = ps_new([M1, 388])
                AeE = ps_new([M1, 388])
                AoE = ps_new([M1, 388])
                for i, dx in enumerate(range(-2, 3)):
                    for src_, pse, pso in ((voddE, AeO, AoO),
                                           (vevenE, AeE, AoE)):
                        p = pse if dx % 2 == 0 else pso
                        MM(p[:], l2A(i), src_[:, 2 + dx:390 + dx],
                           dx in (-2, -1), dx in (1, 2))
                AeES = wpool.tile([M1, 388], F32, tag='AeES')
                AoES = wpool.tile([M1, 388], F32, tag='AoES')
                act(AeES[:], AeE[:], AF.Copy)
                act(AoES[:], AoE[:], AF.Copy)

                hfp = ps_new([M2, 384])
                cser('hf', hfp)
                dctp = ps_new([M2, 384])
                cser('dct', dctp)
                hf2 = tmp('hf2', dt=BF16)
                dc2 = tmp('dc2', dt=BF16)
                act(hf2[:], hfp[:], AF.Square)
                act(dc2[:], dctp[:], AF.Square)
                hbq = tmp('hbq', dt=BF16)
                nc.vector.tensor_add(hbq[:], hf2[:], dc2[:])
                act(stg('highband'), hbq[:], AF.Sqrt, bias=EPS)

                cbp = ps_new([M2, 384])
                cser('cb', cbp)
                act(stg('cb_e'), cbp[:], AF.Abs)
                shp = ps_new([M2, 384])
                cser('sh', shp)
                svp = ps_new([M2, 384])
                cser('sv', svp)
                nc.vector.tensor_copy(stg('sh'), shp[:])
                nc.vector.tensor_copy(stg('sv'), svp[:])
                sxp = ps_new([M2, 384])
                cser('sinx', sxp)
                syp = ps_new([M2, 384])
                cser('siny', syp)
                act(stg('sx'), sxp[:], AF.Copy)
                act(stg('sy'), syp[:], AF.Copy)
                pxp = ps_new([M2, 384])
                cser('phx', pxp)
                pyp = ps_new([M2, 384])
                cser('phy', pyp)
                px2 = tmp('px2', dt=BF16)
                py2 = tmp('py2', dt=BF16)
                act(px2[:], pxp[:], AF.Square)
                act(py2[:], pyp[:], AF.Square)
                phq = tmp('phq', dt=BF16)
                nc.vector.tensor_add(phq[:], px2[:], py2[:])
                act(stg('phase_e'), phq[:], AF.Sqrt, bias=EPS)

                g45p = ps_new([M2, 384])
                cser('g45', g45p)
                g135p = ps_new([M2, 384])
                cser('g135', g135p)
                o1 = tmp('o1', dt=BF16)
                o2 = tmp('o2', dt=BF16)
                act(o1[:], g45p[:], AF.Square)
                act(o2[:], g135p[:], AF.Square)
                oq = tmp('oq', dt=BF16)
                nc.vector.tensor_add(oq[:], o1[:], o2[:])
                act(stg('orient_e'), oq[:], AF.Sqrt, bias=EPS)

                ghp = ps_new([M2, 384])
                cser('gh', ghp)
                gvp = ps_new([M2, 384])
                cser('gv', gvp)
                ghS = tmp('ghS')
                act(ghS[:], ghp[:], AF.Copy)
                tdg = tmp('tdg')
                nc.vector.scalar_tensor_tensor(tdg[:], gvp[:], -1.0,
                                               ghS[:], AL.mult, AL.add)
                nc.vector.scalar_tensor_tensor(stg('dgc'), tdg[:], -1.0,
                                               tdg[:], AL.mult, AL.max)

                gxc = ps_new([M2, 384])
                cser('gxC', gxc)
                gyc = ps_new([M2, 384])
                cser('gyC', gyc)
                act(stg('gx'), gxc[:], AF.Copy)
                act(stg('gy'), gyc[:], AF.Copy)
                sqx = tmp('sqx', dt=BF16)
                sqy = tmp('sqy', dt=BF16)
                act(sqx[:], gxc[:], AF.Square)
                act(sqy[:], gyc[:], AF.Square)
                ssq = tmp('ssq', dt=BF16)
                nc.vector.tensor_add(ssq[:], sqx[:], sqy[:])
                act(stg('grad_mag'), ssq[:], AF.Sqrt, bias=EPS)

                gdm = ps_new([M2, 384])
                cser('gdm', gdm)
                gda = ps_new([M2, 384])
                cser('gda', gda)
                smd = ps_new([M2, 384])
                cser('sumd', smd)
                nc.vector.tensor_copy(stg('gdm'), gdm[:])
                nc.vector.tensor_copy(stg('gda'), gda[:])
                sq1 = tmp('sq1', dt=BF16)
                sq2 = tmp('sq2', dt=BF16)
                act(sq1[:], gdm[:], AF.Square)
                act(sq2[:], gda[:], AF.Square)
                qa = tmp('qa', dt=BF16)
                qb = tmp('qb', dt=BF16)
                nc.vector.tensor_add(qa[:], ssq[:], sq1[:])
                nc.vector.tensor_add(qb[:], qa[:], sq2[:])
                msq = tmp('msq', dt=BF16)
                act(msq[:], smd[:], AF.Square, scale=0.25)
                nc.vector.scalar_tensor_tensor(stg('dir_var'), qb[:], 0.25,
                                               msq[:], AL.mult, AL.subtract)

                lap = ps_new([M2, 384])
                cser('lap', lap)
                hdp = ps_new([M2, 384])
                cser('hd', hdp)
                hxyp = ps_new([M2, 384])
                cser('hxy', hxyp)
                nc.vector.tensor_copy(stg('lap'), lap[:])
                hd2 = tmp('hd2', dt=BF16)
                hxy2 = tmp('hxy2', dt=BF16)
                act(hd2[:], hdp[:], AF.Square, scale=0.5)
                act(hxy2[:], hxyp[:], AF.Square)
                hq = tmp('hq', dt=BF16)
                nc.vector.tensor_add(hq[:], hd2[:], hxy2[:])
                hs = tmp('hs')
                act(hs[:], hq[:], AF.Sqrt, bias=EPS)
                nc.vector.scalar_tensor_tensor(stg('lam_max'), lap[:], 0.5,
                                               hs[:], AL.mult, AL.add)
                nc.vector.scalar_tensor_tensor(stg('lam_min'), lap[:], 0.5,
                                               hs[:], AL.mult, AL.subtract)

                v3a = ps_new([M1, 392])
                vser('V3a', v3a)
                v3b = ps_new([M1, 392])
                vser('V3b', v3b)
                gx = wpool.tile([M1, 388], F32, tag='gx')
                gy = wpool.tile([M1, 388], F32, tag='gy')
                hconv(gx[:], [(v3a[:, 2 + dx:390 + dx], w) for dx, w in gx_w],
                      (hA[:, :388], hB[:, :388]))
                hconv(gy[:], [(v3b[:, 2 + dx:390 + dx], w) for dx, w in gy_w],
                      (hA[:, :388], hB[:, :388]))
                gx2 = wpool.tile([M1, 388], BF16, tag='gx2')
                gy2 = wpool.tile([M1, 388], BF16, tag='gy2')
                gxy = wpool.tile([M1, 388], F32, tag='gxy')
                gxyF = wpool.tile([M1, 388], BF16, tag='gxyF')
                act(gx2[:], gx[:], AF.Square)
                act(gy2[:], gy[:], AF.Square)
                nc.vector.tensor_mul(gxy[:], gx[:], gy[:])
                rsg = rsg_t[0:M1, t:t + 1]
                if cbi == 0:
                    nc.vector.tensor_scalar(gxyF[:, 0:2], gxy[:, 0:2], rsg,
                                            -1.0, AL.mult, AL.mult)
                    nc.vector.tensor_scalar(gxyF[:, 2:388], gxy[:, 2:388],
                                            rsg, None, AL.mult)
                else:
                    nc.vector.tensor_scalar(gxyF[:, 0:386], gxy[:, 0:386],
                                            rsg, None, AL.mult)
                    nc.vector.tensor_scalar(gxyF[:, 386:388], gxy[:, 386:388],
                                            rsg, -1.0, AL.mult, AL.mult)

                gcr = ps_new([M2, 384])
                cser('gcross', gcr)
                bayc = ps_new([M2, 384])
                cser('bayC', bayc)
                bayS = tmp('bayS')
                act(bayS[:], bayc[:], AF.Copy)
                tgi = tmp('tgi')
                nc.vector.scalar_tensor_tensor(
                    tgi[:], gcr[:], -1.0, bayS[:], AL.mult, AL.add)
                nc.vector.tensor_mul(stg('gir'), tgi[:],
                                     mask_t[0:96, 1, c0 + 4:c0 + 388])

                ev = np.s_[:, 0::2]
                od = np.s_[:, 1::2]
                fr = wpool.tile([M1, 388], F32, tag='fr')
                fb = wpool.tile([M1, 388], F32, tag='fb')
                fg = wpool.tile([M1, 388], F32, tag='fg')
                IVR, IVB, IVG = pv(0, 2, 390, 100), pv(1, 2, 390, 100), \
                    pv(2, 2, 390, 100)
                IVGR, IVGB = pv(3, 2, 390, 100), pv(4, 2, 390, 100)
                nc.vector.tensor_mul(fr[ev], AeO[ev], IVR[ev])
                nc.vector.tensor_mul(fr[od], AoO[od], IVR[od])
                nc.vector.tensor_mul(fb[ev], AoE[ev], IVB[ev])
                nc.vector.tensor_mul(fb[od], AeES[od], IVB[od])
                tg = tmp('tg', (M1, 388))
                nc.vector.tensor_add(tg[ev], AoO[ev], AeES[ev])
                nc.vector.tensor_add(tg[od], AeO[od], AoES[od])
                nc.vector.tensor_mul(fg[ev], tg[ev], IVG[ev])
                nc.vector.tensor_mul(fg[od], tg[od], IVG[od])
                fgr = tmp('fgr', (M1, 388))
                fgb = tmp('fgb', (M1, 388))
                nc.vector.tensor_mul(fgr[ev], AoO[ev], IVGR[ev])
                nc.vector.tensor_mul(fgr[od], AeO[od], IVGR[od])
                nc.vector.tensor_mul(fgb[ev], AeES[ev], IVGB[ev])
                nc.vector.tensor_mul(fgb[od], AoES[od], IVGB[od])
                gpdE = wpool.tile([M1, 388], F32, tag='gpdE')
                nc.vector.tensor_sub(gpdE[:], fgr[:], fgb[:])

                rg = wpool.tile([M1, 388], F32, tag='rg')
                bg = wpool.tile([M1, 388], F32, tag='bg')
                nc.vector.tensor_sub(rg[:], fr[:], fg[:])
                nc.vector.tensor_sub(bg[:], fb[:], fg[:])
                rg2 = wpool.tile([M1, 388], BF16, tag='rg2')
                bg2 = wpool.tile([M1, 388], BF16, tag='bg2')
                act(rg2[:], rg[:], AF.Square)
                act(bg2[:], bg[:], AF.Square)
                rgB = wpool.tile([M1, 388], BF16, tag='rgB')
                bgB = wpool.tile([M1, 388], BF16, tag='bgB')
                nc.vector.tensor_copy(rgB[:], rg[:])
                nc.vector.tensor_copy(bgB[:], bg[:])
                cq = tmp('cq', (M1, 388), dt=BF16)
                nc.vector.tensor_add(cq[:], rg2[:], bg2[:])
                chromE = wpool.tile([M1, 388], F32, tag='chromE')
                act(chromE[:], cq[:], AF.Sqrt, bias=EPS)

                # box + cdv
                bx = {}
                for nm, src_ in (('m1r', rgB), ('m1b', bgB),
                                 ('m2r', rg2), ('m2b', bg2)):
                    p = ps_new([M2, 384])
                    for i, dx in enumerate(range(-2, 3)):
                        MM(p[:], l2box(), src_[:, 2 + dx:386 + dx],
                           i == 0, i == 4)
                    bx[nm] = p
                q1 = tmp('q1')
                q2 = tmp('q2')
                act(q1[:], bx['m1r'][:], AF.Square)
                act(q2[:], bx['m1b'][:], AF.Square)
                v1 = tmp('v1')
                v2 = tmp('v2')
                stt(v1[:], q1[:], -1.0, bx['m2r'][:])
                stt(v2[:], q2[:], -1.0, bx['m2b'][:])
                v1m = tmp('v1m')
                v2m = tmp('v2m')
                nc.vector.tensor_scalar(v1m[:], v1[:], 0.0, None, AL.max)
                nc.vector.tensor_scalar(v2m[:], v2[:], 0.0, None, AL.max)
                nc.vector.tensor_add(stg('cdv'), v1m[:], v2m[:])

                # ============ ext gradients -> J -> aniso
                jps = {nm: ps_new([M2, 384]) for nm in
                       ('Jxx', 'Jyy', 'Jxy')}
                jsrc = {'Jxx': gx2, 'Jyy': gy2, 'Jxy': gxyF}
                for i, dx in enumerate(range(-2, 3)):
                    for nm in ('Jxx', 'Jyy', 'Jxy'):
                        MM(jps[nm][:], l2J(i),
                           jsrc[nm][:, 2 + dx:386 + dx], i == 0, i == 4)
                jyyS = tmp('jyyS')
                act(jyyS[:], jps['Jyy'][:], AF.Copy)
                dj = tmp('dj')
                sm = tmp('sm')
                nc.vector.scalar_tensor_tensor(dj[:], jps['Jxx'][:], 1.0,
                                               jyyS[:], AL.mult, AL.subtract)
                nc.vector.scalar_tensor_tensor(sm[:], jps['Jxx'][:], 1.0,
                                               jyyS[:], AL.mult, AL.add)
                dj2 = tmp('dj2', dt=BF16)
                jxy2 = tmp('jxy2', dt=BF16)
                act(dj2[:], dj[:], AF.Square, scale=0.5)
                act(jxy2[:], jps['Jxy'][:], AF.Square)
                qj = tmp('qj', dt=BF16)
                nc.vector.tensor_add(qj[:], dj2[:], jxy2[:])
                anum = tmp('anum')
                act(anum[:], qj[:], AF.Sqrt, bias=4.0 * EPS, scale=4.0)
                sme = tmp('sme')
                nc.vector.tensor_scalar(sme[:], sm[:], EPS, None, AL.add)
                rec = tmp('rec')
                nc.vector.reciprocal(rec[:], sme[:])
                nc.vector.tensor_mul(stg('aniso'), anum[:], rec[:])

                # ============ central channels
                nc.gpsimd.dma_start(
                    out_d[11:14, r0:r0 + 96, c0:c0 + 384]
                    .rearrange('n p w -> p n w'),
                    mask_t[0:96, 0:3, c0 + 4:c0 + 388])

                # ============ output DMAs
                nc.gpsimd.dma_start(
                    out_d[0:11, r0:r0 + 96, c0:c0 + 384]
                    .rearrange('n p w -> p n w'), stA[:])
                # stB: [cdv, cb_e, sh, sv, phase, sx, sy, hb, gir, dgc]
                nc.gpsimd.dma_start(
                    out_d[20:24, r0:r0 + 96, c0:c0 + 384]
                    .rearrange('n p w -> p n w'), stB[:, 0:4, :])
                nc.gpsimd.dma_start(
                    out_d[24:26, r0:r0 + 96, c0:c0 + 384]
                    .rearrange('n p w -> p n w'), stB[:, 2:4, :])
                nc.gpsimd.dma_start(
                    out_d[26:30, r0:r0 + 96, c0:c0 + 384]
                    .rearrange('n p w -> p n w'), stB[:, 4:8, :])
                nc.gpsimd.dma_start(
                    out_d[17:19, r0:r0 + 96, c0:c0 + 384]
                    .rearrange('n p w -> p n w'), stB[:, 8:10, :])
                CENV = np.s_[2:98, 2:386]
                nc.gpsimd.dma_start(out_d[14, r0:r0 + 96, c0:c0 + 384],
                                  gpdE[CENV])
                nc.gpsimd.dma_start(out_d[15, r0:r0 + 96, c0:c0 + 384],
                                  rg[CENV])
                nc.gpsimd.dma_start(out_d[16, r0:r0 + 96, c0:c0 + 384],
                                  bg[CENV])
                nc.gpsimd.dma_start(out_d[19, r0:r0 + 96, c0:c0 + 384],
                                  chromE[CENV])

    nc.compile()
    return nc, lhs1_np, (a_mats, j_mats, box_mat), n1


_STATE = {}


def _get_state():
    if 'nc' not in _STATE:
        nc, lhs1_np, (a_mats, j_mats, box_mat), n1 = build_nc()
        lhs1_pack = np.ascontiguousarray(
            lhs1_np.transpose(1, 0, 2).reshape(K1, n1 * M1))
        lhs2_pack = np.concatenate(
            [a_mats.transpose(1, 0, 2).reshape(K2, 500),
             j_mats.transpose(1, 0, 2).reshape(K2, 480),
             box_mat], axis=1).astype(np.float32)
        import ml_dtypes
        pats, masks = build_patterns()
        pat_pack = np.ascontiguousarray(
            pats.transpose(1, 0, 2).reshape(128, 5 * 776))
        mask_pack = np.ascontiguousarray(
            masks.transpose(1, 0, 2).reshape(128, 3 * 776)
            .astype(ml_dtypes.bfloat16))
        import ml_dtypes
        _STATE.update(nc=nc,
                      lhs1=np.ascontiguousarray(
                          lhs1_pack.astype(ml_dtypes.bfloat16)),
                      lhs2=np.ascontiguousarray(
                          lhs2_pack.astype(ml_dtypes.bfloat16)),
                      pats=pat_pack, masks=mask_pack,
                      rsg=[np.ascontiguousarray(build_rowsign(0)),
                           np.ascontiguousarray(build_rowsign(1))])
    return _STATE


def _run(bayer, trace=False, **kw):
    st = _get_state()
    bayer = np.ascontiguousarray(np.asarray(bayer, dtype=np.float32))
    in_maps = []
    for core in range(8):
        b, h = core // 2, core % 2
        Pimg = np.pad(bayer[b, 0], 4, mode='reflect')
        import ml_dtypes
        bp = np.ascontiguousarray(
            Pimg[h * 384:h * 384 + 392, :].astype(ml_dtypes.bfloat16))
        in_maps.append({'bayer_pad': bp, 'lhs1': st['lhs1'],
                        'lhs2': st['lhs2'], 'pats': st['pats'],
                        'masks': st['masks'], 'rowsgn': st['rsg'][h]})
    res = run_bass_kernel_spmd(st['nc'], in_maps, core_ids=list(range(8)),
                               trace=trace, **kw)
    out = np.empty((4, 30, 768, 768), np.float32)
    for core in range(8):
        b, h = core // 2, core % 2
        out[b, :, h * 384:(h + 1) * 384, :] = res.results[core]['out']
    return out, res


def kernel(bayer):
    out, _ = _run(bayer, trace=False)
    return out



# revision 2
# speedup vs baseline: 1.0428x; 1.0428x over previous
"""Trainium2 Bass kernel for nn_BayerFeatureExtractor (v2: fp8 DoubleRow).

Input:  bayer [4, 1, 768, 768] f32.  Output: [4, 30, 768, 768] f32.

Sharding: data-parallel over 8 cores: core i handles batch b = i//2,
row-half h = i%2 (output rows [h*384, (h+1)*384)).

Per-core: 4 row-tiles (96 out rows) x 2 col-blocks (384 out cols).
All convolutions are banded matmuls contracting over image rows.
Precision-tolerant banks (k5 texture bank, gdm/gda/sumd/hxy/gcross/hf,
box5) run as fp8(e4m3) DoubleRow, two kernel-columns per pass at 0.5
cyc/col: the rhs is a [K, 2, N] view over a DUPLICATED fp8 tile whose
copies sit at an even gap so the k-tile j-stride is 4/16B aligned
(odd/unaligned strides and overlapping views crash the hw; validated
by probes).  All dx pairs are (dx, dx+2).  Precision-critical banks
(gx/gy, lap, hd, gh-gv, fills A-series, J) stay bf16.

Lane discipline: engines cannot shift partitions (start partition must
be 0/32/64/96), so ext-grid results ([100, 388], lane x = out row x-2)
cross to the central grid ([96, 384], lane = out row) only via DMA:
ext channels are packed into extG [100, 5, 388] / shiftE [100, 2, 388]
staging tiles and DMA'd (to DRAM directly, or to a central SBUF tile
for grad_mag/chroma/dir_var inputs).  The bayer identity tap needed by
gir is folded into the gcross kernel (gcross - delta).  aniso's +EPS on
(Jxx+Jyy) is a 1-partition eps-row matmul pass appended to the Jxx
series.  Central channels stage in stG [96, 21, 384] bf16 -> 3 HWDGE
DMAs per block; masks (ch 11-13) are one whole-core DMA.
"""
import math
import os
import sys
from contextlib import ExitStack

import numpy as np

for _p in ('/opt/trn_rl_repo', '/root/.axon_site/_ro/trn_rl_repo'):
    if os.path.isdir(_p) and _p not in sys.path:
        sys.path.insert(0, _p)

import concourse.bass as bass
import concourse.bacc as bacc
import concourse.mybir as mybir
import concourse.tile as tile
from concourse.ap import AP
from concourse.bass_utils import run_bass_kernel_spmd

F32 = mybir.dt.float32
BF16 = mybir.dt.bfloat16
FP8 = mybir.dt.float8e4
AL = mybir.AluOpType
AF = mybir.ActivationFunctionType
DRMODE = mybir.MatmulPerfMode.DoubleRow

EPS = 1e-6
K1, M1 = 104, 100    # ext contraction / rows
K2 = 100             # stage-2 contraction (= M1)
M2 = 96              # central rows

# ---------------------------------------------------------------------------
# constant kernels (identical math to reference._build_kernels)
# ---------------------------------------------------------------------------


def _norm(k):
    k = k - k.mean()
    return (k / max(float(np.abs(k).sum()), 1e-6)).astype(np.float32)


def _gabor(theta, size=5, sigma=1.1, wavelength=3.0, gamma=0.65):
    r = size // 2
    c = np.arange(-r, r + 1, dtype=np.float32)
    yy, xx = np.meshgrid(c, c, indexing='ij')
    xt = xx * math.cos(theta) + yy * math.sin(theta)
    yt = -xx * math.sin(theta) + yy * math.cos(theta)
    env = np.exp(-(xt ** 2 + (gamma * yt) ** 2) / (2.0 * sigma * sigma))
    return _norm(env * np.cos(2.0 * math.pi * xt / wavelength))


def _dct(size=5, u=2, v=2):
    c = np.arange(size, dtype=np.float32)
    return _norm(np.outer(np.cos(math.pi * (c + 0.5) * v / size),
                          np.cos(math.pi * (c + 0.5) * u / size)))


def build_kernels():
    f32 = np.float32
    k3 = np.stack([
        _norm(np.array([[-1, 0, 1], [-2, 0, 2], [-1, 0, 1]], f32)),
        _norm(np.array([[-1, -2, -1], [0, 0, 0], [1, 2, 1]], f32)),
        _norm(np.array([[-2, -1, 0], [-1, 0, 1], [0, 1, 2]], f32)),
        _norm(np.array([[0, 1, 2], [-1, 0, 1], [-2, -1, 0]], f32)),
        np.array([[0, 1, 0], [1, -4, 1], [0, 1, 0]], f32),
        np.array([[0, 0, 0], [1, -2, 1], [0, 0, 0]], f32),
        np.array([[0, 1, 0], [0, -2, 0], [0, 1, 0]], f32),
        np.array([[1, 0, -1], [0, 0, 0], [-1, 0, 1]], f32) / 4.0,
        np.array([[0, .25, 0], [.25, 0, .25], [0, .25, 0]], f32),
        _norm(np.array([[1, -2, 1], [-2, 4, -2], [1, -2, 1]], f32)),
    ])
    ii, jj = np.indices((5, 5))
    s = np.sin(2.0 * math.pi * np.arange(5, dtype=f32) / 5.0)
    c = np.cos(2.0 * math.pi * np.arange(5, dtype=f32) / 5.0)
    k5 = np.stack([
        _norm(((-1.0) ** (ii + jj)).astype(f32)),   # cb
        _norm(((-1.0) ** jj).astype(f32)),          # sh
        _norm(((-1.0) ** ii).astype(f32)),          # sv
        _norm(np.tile(s, (5, 1))),                  # sinx
        _norm(np.tile(s.reshape(5, 1), (1, 5))),    # siny
        _norm(np.tile(c, (5, 1))),                  # phx
        _norm(np.tile(c.reshape(5, 1), (1, 5))),    # phy
        _gabor(math.pi / 4.0),                      # g45
        _gabor(3.0 * math.pi / 4.0),                # g135
        _dct(),                                     # dct
    ])
    ha = np.array([-0.25, 0.5, 0.5, 0.5, -0.25], f32)
    t5 = np.array([1, 2, 3, 2, 1], f32) / 9.0
    return k3, k5, ha, t5


def pad5(col3):
    z = np.zeros(5, np.float32)
    z[1:4] = np.asarray(col3, np.float32)
    return z


def banded_ext(col5):
    B = np.zeros((K1, M1), np.float32)
    for x in range(M1):
        for dy in range(-2, 3):
            k = x + 2 + dy
            if 0 <= k < K1:
                B[k, x] = col5[dy + 2]
    return B


def banded_cen(col5):
    B = np.zeros((K1, M2), np.float32)
    for m in range(M2):
        for dy in range(-2, 3):
            k = m + 4 + dy
            if 0 <= k < K1:
                B[k, m] = col5[dy + 2]
    return B


def banded_s2(col5, scale=1.0):
    B = np.zeros((K2, M2), np.float32)
    for y in range(M2):
        for dy in range(-2, 3):
            k = y + 2 + dy
            if 0 <= k < K2:
                B[k, y] = col5[dy + 2] * scale
    return B


def fp8_quant(x):
    import ml_dtypes
    return np.asarray(x, np.float32).astype(
        ml_dtypes.float8_e4m3).astype(np.float32)


def best_ws(kern):
    k = np.asarray(kern, np.float32)
    best = None
    for e in range(-30, 31):
        s = 1.05 ** e
        err = float(np.abs(fp8_quant(k * s) / s - k).sum())
        if best is None or err < best[0]:
            best = (err, s)
    return best[1]


def cols_of(kern):
    k = np.asarray(kern, np.float32)
    if k.shape[0] == 3:
        kk = np.zeros((5, 5), np.float32)
        kk[1:4, 1:4] = k
        k = kk
    out = []
    for dx in range(-2, 3):
        col = k[:, dx + 2]
        if np.any(col != 0):
            out.append((dx, col.astype(np.float32)))
    return out


def make_pairs(dxs):
    """Pairs at distance exactly 2 (even j-stride gap required by hw);
    lone dx becomes a zero-padded pass, biased toward small dx0 so the
    padded k-tile window stays in-bounds."""
    rest = sorted(dxs, reverse=True)
    pairs = []
    while rest:
        x = rest.pop(0)
        if x - 2 in rest:
            rest.remove(x - 2)
            pairs.append((x - 2, x))
        else:
            pairs.append((x, None))
    return list(reversed(pairs))


# ---------------------------------------------------------------------------
# weight packs
# ---------------------------------------------------------------------------

def build_packs():
    k3, k5, ha, t5 = build_kernels()

    mats = []
    bidx = {}

    def addb(name, mlist, dxs=None):
        bidx[name] = (len(mats), len(mlist))
        if dxs is not None:
            bidx[name + '_dx'] = dxs
        mats.extend(mlist)

    for nm, kern in (('gx', k3[0]), ('gy', k3[1])):
        coll = cols_of(kern)
        addb(nm, [banded_ext(c) for dx, c in coll], [dx for dx, c in coll])
    for nm, kern in (('lap', k3[4]), ('hd', k3[5] - k3[6])):
        coll = cols_of(kern)
        addb(nm, [banded_cen(c) for dx, c in coll], [dx for dx, c in coll])
    I5 = pad5([0, 1, 0])
    addb('ghgv0', [banded_cen(I5) * ha[dx + 2] for dx in (-2, -1, 1, 2)],
         [-2, -1, 1, 2])
    addb('ghgvC', [banded_cen(I5) * ha[2] - banded_cen(ha)])
    gcmi = k3[8].copy()
    gcmi[1, 1] -= 1.0   # gcross - delta: psum = gcross*b - b
    coll = cols_of(gcmi)
    addb('gcmi', [banded_cen(c) for dx, c in coll], [dx for dx, c in coll])
    kk = (np.arange(K1) % 2).astype(np.float32)
    Bod = banded_ext(t5) * kk[:, None]
    Bev = banded_ext(t5) * (1.0 - kk)[:, None]
    fm = []
    fa_start = len(mats)
    for aname, Ba in (('O', Bod), ('E', Bev)):
        for dx in range(-2, 3):
            grp = ('Ae' if dx % 2 == 0 else 'Ao') + aname
            fm.append((grp, dx))
            mats.append(Ba * t5[dx + 2])
    bidx['fillsA'] = (fa_start, 10)
    bidx['fillsA_meta'] = fm
    lhs1 = np.zeros((K1, len(mats) * M1), np.float32)
    for i, m in enumerate(mats):
        lhs1[:, i * M1:i * M1 + m.shape[1]] = m

    # stage-2 bf16: J series [K2, 5, 96]
    lhs2 = np.concatenate(
        [banded_s2(t5, t5[dx + 2]) for dx in range(-2, 3)], axis=1)

    # fp8 stage-1 DoubleRow
    f8kern = {'gdm': k3[2], 'gda': k3[3],
              'sumd': k3[0] + k3[1] + k3[2] + k3[3],
              'hxy': k3[7], 'hf': k3[9],
              'cb': k5[0], 'sh': k5[1], 'sv': k5[2], 'sinx': k5[3],
              'siny': k5[4], 'phx': k5[5], 'phy': k5[6], 'g45': k5[7],
              'g135': k5[8], 'dct': k5[9]}
    f8packs = []
    f8idx = {}
    for nm, kern in f8kern.items():
        ws = 1.0 if nm in ('sinx', 'siny') else best_ws(kern)
        coll = cols_of(kern)
        cold = {dx: c for dx, c in coll}
        pl = []
        start = len(f8packs)
        for dx0, dx1 in make_pairs([dx for dx, c in coll]):
            W = np.zeros((K1, 2, M2), np.float32)
            W[:, 0, :] = banded_cen(cold[dx0]) * ws
            if dx1 is not None:
                W[:, 1, :] = banded_cen(cold[dx1]) * ws
            f8packs.append(W)
            pl.append(dx0)
        f8idx[nm] = (start, pl, ws)
    lhs8 = np.zeros((K1, len(f8packs) * 2 * M2), np.float32)
    for i, W in enumerate(f8packs):
        lhs8[:, i * 2 * M2:(i + 1) * 2 * M2] = W.reshape(K1, 2 * M2)

    # fp8 stage-2 box: taps 0.25*0.25 exact; true scale 16/25
    BOX_SCALE = 16.0 / 25.0
    ones5 = np.ones(5, np.float32)
    box_pairs = make_pairs(range(-2, 3))
    box8 = np.zeros((K2, len(box_pairs) * 2 * M2), np.float32)
    box_meta = []
    for i, (dx0, dx1) in enumerate(box_pairs):
        W = np.zeros((K2, 2, M2), np.float32)
        W[:, 0, :] = banded_s2(ones5 * 0.25, 0.25)
        if dx1 is not None:
            W[:, 1, :] = banded_s2(ones5 * 0.25, 0.25)
        box8[:, i * 2 * M2:(i + 1) * 2 * M2] = W.reshape(K2, 2 * M2)
        box_meta.append(dx0)

    return dict(lhs1=lhs1, bidx=bidx, lhs2=lhs2, lhs8=lhs8, f8idx=f8idx,
                n8=len(f8packs), box8=box8, box_meta=box_meta,
                BOX_SCALE=BOX_SCALE, nb=len(mats))


def build_patterns():
    t5 = np.array([1, 2, 3, 2, 1], np.float32) / 9.0

    def mfun(ch, rp, cp):
        return {
            'r': float(rp == 1 and cp == 0),
            'b': float(rp == 0 and cp == 1),
            'gr': float(rp == 1 and cp == 1),
            'gb': float(rp == 0 and cp == 0),
            'g': float((rp == 1 and cp == 1) or (rp == 0 and cp == 0)),
        }[ch]

    P, W = 128, 776
    pp = np.arange(P)[:, None] % 2
    cc = np.arange(W)[None, :] % 2
    pats = []
    for ch in ['r', 'b', 'g', 'gr', 'gb']:
        v = np.zeros((2, 2), np.float32)
        for rp in range(2):
            for cp in range(2):
                d = sum(t5[dy + 2] * t5[dx + 2]
                        * mfun(ch, (rp + dy) % 2, (cp + dx) % 2)
                        for dy in range(-2, 3) for dx in range(-2, 3))
                v[rp, cp] = 1.0 / max(d, EPS)
        pats.append(v[pp, cc].astype(np.float32))
    gmask = np.array([[mfun('g', rp, cp) for cp in range(2)]
                      for rp in range(2)], np.float32)[pp, cc]
    return np.stack(pats), gmask.astype(np.float32)


def build_maskout():
    er = (np.arange(384) % 2 == 0).astype(np.float32)[:, None]
    ec = (np.arange(768) % 2 == 0).astype(np.float32)[None, :]
    gb_m = er * ec
    b_m = er * (1.0 - ec)
    r_m = (1.0 - er) * ec
    gr_m = (1.0 - er) * (1.0 - ec)
    g_m = gr_m + gb_m
    return np.stack([r_m, g_m, b_m])


def build_rowsign(h):
    sg = np.ones((4, M1), np.float32)
    for t in range(4):
        for x in range(M1):
            r = h * 384 + 96 * t - 2 + x
            if r < 0 or r >= 768:
                sg[t, x] = -1.0
    return sg.T.copy()  # [100, 4]


# central staging slots: 0..8 = ch2..10; 9..10 = ch17..18; 11..20 = ch20..29;
# 21 = ch19 (chroma, written by its own DMA)
CSLOT = {'gdm': 0, 'gda': 1, 'grad_mag': 2, 'lap': 3, 'lam_max': 4,
         'lam_min': 5, 'aniso': 6, 'dir_var': 7, 'orient_e': 8,
         'gir': 9, 'dgc': 10, 'cdv': 11, 'cb_e': 12, 'sh': 13, 'sv': 14,
         'ax': 15, 'ay': 16, 'phase_e': 17, 'sx': 18, 'sy': 19,
         'highband': 20, 'chroma': 21}
# ext staging slots (extG): gx, gy -> ch0,1; gpd, rg, bg -> ch14,15,16
ESLOT = {'gx': 0, 'gy': 1, 'gpd': 2, 'rg': 3, 'bg': 4, 'chroma_sq': 5}


# ---------------------------------------------------------------------------
# kernel builder
# ---------------------------------------------------------------------------

def build_nc():
    packs = build_packs()
    bidx = packs['bidx']
    f8idx = packs['f8idx']
    BOXS = packs['BOX_SCALE']
    NB = packs['nb']
    N8 = packs['n8']
    NBX = len(packs['box_meta'])

    nc = bacc.Bacc(None, target_bir_lowering=False)
    bay_d = nc.dram_tensor('bayer_pad', [392, 776], BF16, kind='ExternalInput')
    bay8_d = nc.dram_tensor('bayer_pad8', [392, 776], FP8,
                            kind='ExternalInput')
    lhs1_d = nc.dram_tensor('lhs1', [K1, NB * M1], BF16, kind='ExternalInput')
    lhs2_d = nc.dram_tensor('lhs2', [K2, 5 * M2], BF16, kind='ExternalInput')
    lhs8_d = nc.dram_tensor('lhs8', [K1, N8 * 2 * M2], FP8,
                            kind='ExternalInput')
    box8_d = nc.dram_tensor('box8', [K2, NBX * 2 * M2], FP8,
                            kind='ExternalInput')
    pat_d = nc.dram_tensor('pats', [128, 5 * 776], BF16, kind='ExternalInput')
    gm_d = nc.dram_tensor('gmask', [128, 776], BF16, kind='ExternalInput')
    rsg_d = nc.dram_tensor('rowsgn', [M1, 4], F32, kind='ExternalInput')
    mo_d = nc.dram_tensor('maskout', [3, 384, 768], BF16,
                          kind='ExternalInput')
    out_d = nc.dram_tensor('out', [30, 384, 768], BF16, kind='ExternalOutput')

    with tile.TileContext(nc) as tc, ExitStack() as ctx:
        cpool = ctx.enter_context(tc.tile_pool(name='const', bufs=1))
        inpool = ctx.enter_context(tc.tile_pool(name='inp', bufs=2))
        wpool = ctx.enter_context(tc.tile_pool(name='work', bufs=2))
        spool = ctx.enter_context(tc.tile_pool(name='stage', bufs=2))
        pspool = ctx.enter_context(
            tc.tile_pool(name='ps', bufs=8, space='PSUM'))

        epsT = cpool.tile([128, 1], F32, tag='epsT', name='epsT')
        eps4T = cpool.tile([128, 1], F32, tag='eps4T', name='eps4T')
        onesW = cpool.tile([1, M2], BF16, tag='onesW', name='onesW')
        epsRow = cpool.tile([1, 384], BF16, tag='epsRow', name='epsRow')
        nc.vector.memset(epsT[:], EPS)
        nc.vector.memset(eps4T[:], 4.0 * EPS)
        nc.vector.memset(onesW[:], 1.0)
        nc.vector.memset(epsRow[:], EPS)
        zeroT = cpool.tile([128, 384], BF16, tag='zeroT', name='zeroT')
        nc.vector.memset(zeroT[:], 0.0)
        lhs1_t = cpool.tile([K1, NB * M1], BF16, tag='lhs1')
        lhs2_t = cpool.tile([K2, 5, M2], BF16, tag='lhs2')
        lhs8_t = cpool.tile([K1, N8, 2, M2], FP8, tag='lhs8')
        box8_t = cpool.tile([K2, NBX, 2, M2], FP8, tag='box8')
        pat_t = cpool.tile([128, 5, 776], BF16, tag='pats')
        gm_t = cpool.tile([128, 776], BF16, tag='gmask')
        rsg_t = cpool.tile([M1, 4], F32, tag='rsg')
        nc.sync.dma_start(lhs1_t[:], lhs1_d[:])
        nc.sync.dma_start(
            lhs8_t[:], lhs8_d[:].rearrange('k (n j m) -> k n j m',
                                           n=N8, j=2))
        nc.sync.dma_start(
            pat_t[:], pat_d[:].rearrange('p (n w) -> p n w', n=5))  # [128,5,2]
        nc.sync.dma_start(rsg_t[:], rsg_d[:])
        nc.sync.dma_start(gm_t[:], gm_d[:])
        nc.sync.dma_start(
            lhs2_t[:], lhs2_d[:].rearrange('k (n m) -> k n m', n=5))
        nc.sync.dma_start(
            box8_t[:], box8_d[:].rearrange('k (n j m) -> k n j m',
                                           n=NBX, j=2))
        nc.sync.dma_start(out_d[11:14, :, :], mo_d[:])

        ctxd = dict(nc=nc, packs=packs, bidx=bidx, f8idx=f8idx, BOXS=BOXS,
                    zeroT=zeroT,
                    lhs1_t=lhs1_t, lhs2_t=lhs2_t, lhs8_t=lhs8_t,
                    box8_t=box8_t, pat_t=pat_t, gm_t=gm_t, rsg_t=rsg_t,
                    epsT=epsT, eps4T=eps4T, onesW=onesW, epsRow=epsRow,
                    out_d=out_d, wpool=wpool, spool=spool, pspool=pspool)

        pending = None
        for t in range(4):
            r0 = 96 * t
            BT = inpool.tile([K1, 776], BF16, tag='BT')
            BT8 = inpool.tile([K1, 2, 782], FP8, tag='BT8')
            nc.sync.dma_start(BT[:], bay_d[r0:r0 + 104, :])
            nc.sync.dma_start(BT8[:, 0, 0:776], bay8_d[r0:r0 + 104, :])
            nc.sync.dma_start(BT8[:, 1, 0:776], bay8_d[r0:r0 + 104, :])
            for cbi in range(2):
                stA = build_block_A(ctxd, t, cbi, BT, BT8)
                if pending is not None:
                    build_block_B(ctxd, pending)
                pending = stA
        build_block_B(ctxd, pending)

    nc.compile()
    return nc, packs


def _helpers(C, t, cbi):
    nc = C['nc']
    lhs1_t = C['lhs1_t']
    bidx = C['bidx']
    epsT, eps4T = C['epsT'], C['eps4T']
    wpool, pspool = C['wpool'], C['pspool']

    def l1(name, i=0, M=M2):
        s, _ = bidx[name]
        return lhs1_t[:, (s + i) * M1:(s + i) * M1 + M]

    def ps_new(shape, tag):
        return pspool.tile(list(shape), F32, tag='ps', name=tag)

    def wt(tag, shape=(M2, 384), dt=BF16):
        return wpool.tile(list(shape), dt, tag=tag, name=tag)

    def act(out, in_, func, bias=0.0, scale=1.0):
        if isinstance(bias, float) and bias != 0.0:
            bt = eps4T if bias == 4.0 * EPS else epsT
            bias = bt[0:out.shape[0], :]
        nc.scalar.activation(out, in_, func, bias=bias, scale=scale)

    def MM(ps, lh, rh, start, stop):
        nc.tensor.matmul(ps, lh, rh, start=start, stop=stop)

    def drview(dup_tile, pre, coff, npart, N, W):
        sl = dup_tile[pre + (slice(coff, coff + N),)]
        return AP(sl.tensor, sl.offset,
                  [list(sl.ap[0]), [W + 2, 2], [1, N]])

    return l1, ps_new, wt, act, MM, drview


def build_block_A(C, t, cbi, BT, BT8):
    """Stage A: all stage-1 matmul series + fills/product vector work.
    Returns state consumed by build_block_B one block later."""
    nc = C['nc']
    packs, bidx, f8idx = C['packs'], C['bidx'], C['f8idx']
    lhs8_t = C['lhs8_t']
    pat_t, gm_t, rsg_t = C['pat_t'], C['gm_t'], C['rsg_t']
    out_d = C['out_d']
    spool = C['spool']
    V = nc.vector
    P = nc.gpsimd
    r0 = 96 * t
    c0 = 384 * cbi
    ev = np.s_[:, 0::2]
    od = np.s_[:, 1::2]
    CENF = np.s_[2:386]
    l1, ps_new, wt, act, MM, drview = _helpers(C, t, cbi)

    def bserE(name, tag):
        p = ps_new([M1, 388], tag)
        dxs = bidx[name + '_dx']
        for i, dx in enumerate(dxs):
            MM(p[:], l1(name, i, M1),
               BT[:, c0 + 2 + dx:c0 + 2 + dx + 388],
               i == 0, i == len(dxs) - 1)
        return p

    def bserC(name, tag, stop=True):
        p = ps_new([M2, 384], tag)
        dxs = bidx[name + '_dx']
        for i, dx in enumerate(dxs):
            MM(p[:], l1(name, i), BT[:, c0 + 4 + dx:c0 + 4 + dx + 384],
               i == 0, stop and i == len(dxs) - 1)
        return p

    def f8ser(name, tag):
        p = ps_new([M2, 384], tag)
        start, pl, ws = f8idx[name]
        for i, dx0 in enumerate(pl):
            rhs = drview(BT8, (slice(0, K1), 0), c0 + 4 + dx0, K1, 384, 782)
            nc.tensor.matmul(p[:], lhs8_t[:, start + i, :, :], rhs,
                             start=(i == 0), stop=(i == len(pl) - 1),
                             perf_mode=DRMODE)
        return p

    stG = spool.tile([M2, 22, 384], BF16, tag='stG')
    extG = spool.tile([M1, 6, 388], BF16, tag='extG')
    shiftC = spool.tile([M2, 2, 384], BF16, tag='shiftC')

    def stg(name):
        return stG[:, CSLOT[name], :]

    def ext(name):
        return extG[:, ESLOT[name], :]

    # ===== PE: ext gradients + fills A-series =====
    gxP = bserE('gx', 'gxP')
    gyP = bserE('gy', 'gyP')
    fa_start, _ = bidx['fillsA']
    fa_meta = bidx['fillsA_meta']
    Aps = {}
    for g in ('AeO', 'AoO', 'AeE', 'AoE'):
        idxs = [i for i, (gg, dx) in enumerate(fa_meta) if gg == g]
        p = ps_new([M1, 388], 'A' + g)
        for j, i in enumerate(idxs):
            dx = fa_meta[i][1]
            MM(p[:], l1('fillsA', i, M1),
               BT[:, c0 + 2 + dx:c0 + 2 + dx + 388],
               j == 0, j == len(idxs) - 1)
        Aps[g] = p

    act(ext('gx'), gxP[:], AF.Copy)
    act(ext('gy'), gyP[:], AF.Copy)
    As = {g: wt('As' + g, (M1, 388)) for g in ('AeE', 'AoE')}
    act(As['AeE'][:], Aps['AeE'][:], AF.Copy)
    act(As['AoE'][:], Aps['AoE'][:], AF.Copy)

    # gradient products (feed J in stage B)
    gx2 = wt('gx2', (M1, 388))
    gy2 = wt('gy2', (M1, 388))
    gxyF = wt('gxyF', (M1, 388))
    V.tensor_mul(gx2[:], ext('gx'), ext('gx'))
    V.tensor_mul(gy2[:], ext('gy'), ext('gy'))
    rsg = rsg_t[0:M1, t:t + 1]
    V.scalar_tensor_tensor(gxyF[:], ext('gx'), rsg, ext('gy'),
                           AL.mult, AL.mult)
    if cbi == 0:
        V.tensor_scalar(gxyF[:, 0:2], gxyF[:, 0:2], -1.0, None, AL.mult)
    else:
        V.tensor_scalar(gxyF[:, 386:388], gxyF[:, 386:388], -1.0, None,
                        AL.mult)
    ssqE = wt('ssqE', (M1, 388))
    V.tensor_add(ssqE[:], gx2[:], gy2[:])
    d2E = wt('d2E', (M1, 388))
    V.tensor_sub(d2E[:], gx2[:], gy2[:])

    # ===== DVE: fills chain =====
    def pv(pi):
        return pat_t[0:M1, pi, c0 + 2:c0 + 2 + 388]

    IVR, IVB, IVG, IVGR, IVGB = pv(0), pv(1), pv(2), pv(3), pv(4)
    fr = wt('fr', (M1, 388))
    fb = wt('fb', (M1, 388))
    tg = wt('tg', (M1, 388))
    fg = wt('fg', (M1, 388))
    fgr = wt('fgr', (M1, 388))
    fgb = wt('fgb', (M1, 388))
    V.tensor_mul(fr[ev], Aps['AeO'][ev], IVR[ev])
    V.tensor_mul(fr[od], Aps['AoO'][od], IVR[od])
    V.tensor_mul(fb[ev], As['AoE'][ev], IVB[ev])
    V.tensor_mul(fb[od], As['AeE'][od], IVB[od])
    V.tensor_add(tg[ev], Aps['AoO'][ev], As['AeE'][ev])
    V.tensor_add(tg[od], Aps['AeO'][od], As['AoE'][od])
    P.tensor_mul(fg[ev], tg[ev], IVG[ev])
    P.tensor_mul(fg[od], tg[od], IVG[od])
    V.tensor_mul(fgr[ev], Aps['AoO'][ev], IVGR[ev])
    V.tensor_mul(fgr[od], Aps['AeO'][od], IVGR[od])
    V.tensor_mul(fgb[ev], As['AeE'][ev], IVGB[ev])
    V.tensor_mul(fgb[od], As['AoE'][od], IVGB[od])
    V.tensor_sub(ext('gpd'), fgr[:], fgb[:])
    V.tensor_sub(ext('rg'), fr[:], fg[:])
    V.tensor_sub(ext('bg'), fb[:], fg[:])

    dup4 = wt('dup4', (M1, 4, 2, 390), FP8)
    P.tensor_copy(dup4[:, 0, 0, 0:388], ext('rg'))
    P.tensor_copy(dup4[:, 1, 0, 0:388], ext('bg'))
    rgsq = wt('rgsq', (M1, 388))
    V.tensor_mul(rgsq[:], ext('rg'), ext('rg'))
    bgsq = wt('bgsq', (M1, 388))
    V.tensor_mul(bgsq[:], ext('bg'), ext('bg'))

    shiftE = wt('shiftE', (M1, 2, 388))
    V.tensor_copy(shiftE[:, 0, :], ssqE[:])
    V.tensor_add(shiftE[:, 1, :], rgsq[:], bgsq[:])
    nc.sync.dma_start(shiftC[:], shiftE[2:98, :, CENF])
    P.tensor_copy(dup4[:, 2, 0, 0:388], rgsq[:])
    P.tensor_copy(dup4[:, 3, 0, 0:388], bgsq[:])
    nc.sync.dma_start(dup4[:, :, 1, 0:388], dup4[:, :, 0, 0:388])

    # ===== PE: fp8 + bf16 central series, drains interleaved =====
    gdmP = f8ser('gdm', 'gdmP')
    V.tensor_copy(stg('gdm'), gdmP[:])
    gdaP = f8ser('gda', 'gdaP')
    V.tensor_copy(stg('gda'), gdaP[:])
    _, _, ws_smd = f8idx['sumd']
    smdP = f8ser('sumd', 'smdP')
    msq = wt('msq')
    act(msq[:], smdP[:], AF.Square, scale=0.25 / ws_smd)
    sq1 = wt('sq1')
    V.tensor_mul(sq1[:], stg('gdm'), stg('gdm'))
    sq2 = wt('sq2')
    V.tensor_mul(sq2[:], stg('gda'), stg('gda'))

    lapP = bserC('lap', 'lapP')
    V.tensor_copy(stg('lap'), lapP[:])
    hdP = bserC('hd', 'hdP')
    _, _, ws_hxy = f8idx['hxy']
    hxyP = f8ser('hxy', 'hxyP')
    hd2 = wt('hd2')
    act(hd2[:], hdP[:], AF.Square, scale=0.5)
    hxy2 = wt('hxy2')
    act(hxy2[:], hxyP[:], AF.Square, scale=1.0 / ws_hxy)
    hq = wt('hq')
    P.tensor_add(hq[:], hd2[:], hxy2[:])
    hs = wt('hs')
    act(hs[:], hq[:], AF.Sqrt, bias=EPS)
    V.scalar_tensor_tensor(stg('lam_max'), lapP[:], 0.5, hs[:], AL.mult,
                           AL.add)
    V.scalar_tensor_tensor(stg('lam_min'), lapP[:], 0.5, hs[:], AL.mult,
                           AL.subtract)

    gcrP = bserC('gcmi', 'gcrP')
    V.scalar_tensor_tensor(stg('gir'), gcrP[:], -1.0,
                           gm_t[0:M2, c0 + 4:c0 + 388], AL.mult, AL.mult)

    ghvP = bserC('ghgv0', 'ghvP', stop=False)
    MM(ghvP[:], l1('ghgvC'), BT[:, c0 + 4:c0 + 4 + 384], False, True)
    act(stg('dgc'), ghvP[:], AF.Abs)

    def sq_pair(nameA, nameB, outname, tagp):
        _, _, wsA = f8idx[nameA]
        _, _, wsB = f8idx[nameB]
        pA = f8ser(nameA, tagp + 'A')
        pB = f8ser(nameB, tagp + 'B')
        a2 = wt(tagp + 'a2')
        act(a2[:], pA[:], AF.Square, scale=1.0 / wsA)
        b2 = wt(tagp + 'b2')
        act(b2[:], pB[:], AF.Square, scale=1.0 / wsB)
        q = wt(tagp + 'q')
        P.tensor_add(q[:], a2[:], b2[:])
        act(stg(outname), q[:], AF.Sqrt, bias=EPS)

    sq_pair('hf', 'dct', 'highband', 'hb')
    sq_pair('phx', 'phy', 'phase_e', 'ph')
    sq_pair('g45', 'g135', 'orient_e', 'oe')

    _, _, ws_cb = f8idx['cb']
    cbP = f8ser('cb', 'cbP')
    act(stg('cb_e'), cbP[:], AF.Abs, scale=1.0 / ws_cb)

    for nm, slot in (('sh', 'sh'), ('sv', 'sv')):
        _, _, ws = f8idx[nm]
        p = f8ser(nm, 'p' + nm)
        act(stg(slot), p[:], AF.Copy, scale=1.0 / ws)
    for nm, slot in (('sinx', 'sx'), ('siny', 'sy')):
        p = f8ser(nm, 'p' + nm)
        V.tensor_copy(stg(slot), p[:])

    # ext channels straight to DRAM
    nc.sync.dma_start(
        out_d[0:2, r0:r0 + 96, c0:c0 + 384].rearrange('n p w -> p n w'),
        extG[2:98, 0:2, CENF])
    nc.sync.dma_start(
        out_d[14:17, r0:r0 + 96, c0:c0 + 384].rearrange('n p w -> p n w'),
        extG[2:98, 2:5, CENF])

    return dict(t=t, cbi=cbi, stG=stG, shiftC=shiftC, dup4=dup4,
                gxyF=gxyF, ssqE=ssqE, d2E=d2E, msq=msq, sq1=sq1, sq2=sq2)


def build_block_B(C, S):
    """Stage B: J + box matmuls and tails for the PREVIOUS block."""
    nc = C['nc']
    packs, f8idx, BOXS = C['packs'], C['f8idx'], C['BOXS']
    lhs2_t, box8_t = C['lhs2_t'], C['box8_t']
    onesW, epsRow = C['onesW'], C['epsRow']
    out_d = C['out_d']
    V = nc.vector
    P = nc.gpsimd
    t, cbi = S['t'], S['cbi']
    r0 = 96 * t
    c0 = 384 * cbi
    l1, ps_new, wt, act, MM, drview = _helpers(C, t, cbi)
    stG, shiftC, dup4 = S['stG'], S['shiftC'], S['dup4']
    gxyF, ssqE, d2E = S['gxyF'], S['ssqE'], S['d2E']
    msq, sq1, sq2 = S['msq'], S['sq1'], S['sq2']

    def stg(name):
        return stG[:, CSLOT[name], :]

    # ===== PE: smooth5 of (gx2-gy2), (gx2+gy2)+eps, gxy =====
    djP = ps_new([M2, 384], 'djP')
    smP = ps_new([M2, 384], 'smP')
    jxyP = ps_new([M2, 384], 'jxyP')
    for i, dx in enumerate(range(-2, 3)):
        MM(djP[:], lhs2_t[:, i, :], d2E[:, 2 + dx:2 + dx + 384],
           i == 0, i == 4)
        MM(jxyP[:], lhs2_t[:, i, :], gxyF[:, 2 + dx:2 + dx + 384],
           i == 0, i == 4)
        MM(smP[:], lhs2_t[:, i, :], ssqE[:, 2 + dx:2 + dx + 384],
           i == 0, False)
    nc.tensor.matmul(smP[:], onesW[:], epsRow[:], start=False, stop=True)

    dj2 = wt('dj2')
    act(dj2[:], djP[:], AF.Square, scale=0.5)
    jxy2 = wt('jxy2')
    act(jxy2[:], jxyP[:], AF.Square)
    qj = wt('qj')
    P.tensor_add(qj[:], dj2[:], jxy2[:])
    anum = wt('anum')
    act(anum[:], qj[:], AF.Sqrt, bias=4.0 * EPS, scale=4.0)
    rec = wt('rec', dt=F32)
    V.reciprocal(rec[:], smP[:])
    P.tensor_mul(stg('aniso'), anum[:], rec[:])

    # ===== PE: box (fp8 DR stage-2) =====
    def boxser(slot, tag):
        p = ps_new([M2, 384], tag)
        for i, dx0 in enumerate(packs['box_meta']):
            rhs = drview(dup4, (slice(0, K2), slot, 0), 2 + dx0,
                         K2, 384, 390)
            nc.tensor.matmul(p[:], box8_t[:, i, :, :], rhs,
                             start=(i == 0),
                             stop=(i == len(packs['box_meta']) - 1),
                             perf_mode=DRMODE)
        return p

    m1r = boxser(0, 'm1r')
    m1b = boxser(1, 'm1b')
    m2r = boxser(2, 'm2r')
    m2b = boxser(3, 'm2b')
    q1 = wt('q1')
    act(q1[:], m1r[:], AF.Square, scale=BOXS)
    q2 = wt('q2')
    act(q2[:], m1b[:], AF.Square, scale=BOXS)
    v1 = wt('v1')
    V.scalar_tensor_tensor(v1[:], m2r[:], BOXS, q1[:], AL.mult, AL.subtract)
    v2 = wt('v2')
    V.scalar_tensor_tensor(v2[:], m2b[:], BOXS, q2[:], AL.mult, AL.subtract)
    v1m = wt('v1m')
    V.tensor_scalar(v1m[:], v1[:], 0.0, None, AL.max)
    v2m = wt('v2m')
    V.tensor_scalar(v2m[:], v2[:], 0.0, None, AL.max)
    P.tensor_add(stg('cdv'), v1m[:], v2m[:])

    # ===== shiftC-dependent tail =====
    act(stg('grad_mag'), shiftC[:, 0, :], AF.Sqrt, bias=EPS)
    act(stg('chroma'), shiftC[:, 1, :], AF.Sqrt, bias=EPS)
    qa = wt('qa')
    P.tensor_add(qa[:], shiftC[:, 0, :], sq1[:])
    qb = wt('qb')
    P.tensor_add(qb[:], qa[:], sq2[:])
    V.scalar_tensor_tensor(stg('dir_var'), qb[:], 0.25, msq[:], AL.mult,
                           AL.subtract)

    # ===== stG output DMAs =====
    nc.sync.dma_start(
        out_d[2:11, r0:r0 + 96, c0:c0 + 384].rearrange('n p w -> p n w'),
        stG[:, 0:9, :])
    nc.sync.dma_start(
        out_d[17:19, r0:r0 + 96, c0:c0 + 384].rearrange('n p w -> p n w'),
        stG[:, 9:11, :])
    nc.sync.dma_start(
        out_d[20:24, r0:r0 + 96, c0:c0 + 384].rearrange('n p w -> p n w'),
        stG[:, 11:15, :])
    nc.sync.dma_start(
        out_d[26:30, r0:r0 + 96, c0:c0 + 384].rearrange('n p w -> p n w'),
        stG[:, 17:21, :])
    nc.sync.dma_start(out_d[19, r0:r0 + 96, c0:c0 + 384],
                      stG[:, CSLOT['chroma'], :])
    nc.sync.dma_start(
        out_d[24:26, r0:r0 + 96, c0:c0 + 384].rearrange('n p w -> p n w'),
        stG[:, CSLOT['sh']:CSLOT['sv'] + 1, :])


# ---------------------------------------------------------------------------
# host wrapper
# ---------------------------------------------------------------------------

_STATE = {}


def _get_state():
    if 'nc' not in _STATE:
        import ml_dtypes
        nc, packs = build_nc()
        pats, gmask = build_patterns()
        _STATE.update(
            nc=nc,
            lhs1=np.ascontiguousarray(
                packs['lhs1'].astype(ml_dtypes.bfloat16)),
            lhs2=np.ascontiguousarray(
                packs['lhs2'].astype(ml_dtypes.bfloat16)),
            lhs8=np.ascontiguousarray(
                packs['lhs8'].astype(ml_dtypes.float8_e4m3)),
            box8=np.ascontiguousarray(
                packs['box8'].astype(ml_dtypes.float8_e4m3)),
            pats=np.ascontiguousarray(
                pats.transpose(1, 0, 2).reshape(128, 5 * 776)
                .astype(ml_dtypes.bfloat16)),
            gmask=np.ascontiguousarray(gmask.astype(ml_dtypes.bfloat16)),
            maskout=np.ascontiguousarray(
                build_maskout().astype(ml_dtypes.bfloat16)),
            rsg=[np.ascontiguousarray(build_rowsign(0)),
                 np.ascontiguousarray(build_rowsign(1))])
    return _STATE


def _run(bayer, trace=False, **kw):
    import ml_dtypes
    st = _get_state()
    bayer = np.ascontiguousarray(np.asarray(bayer, dtype=np.float32))
    in_maps = []
    for core in range(8):
        b, h = core // 2, core % 2
        Pimg = np.pad(bayer[b, 0], 4, mode='reflect')
        sl = Pimg[h * 384:h * 384 + 392, :]
        bp = np.ascontiguousarray(sl.astype(ml_dtypes.bfloat16))
        bp8 = np.ascontiguousarray(sl.astype(ml_dtypes.float8_e4m3))
        in_maps.append({'bayer_pad': bp, 'bayer_pad8': bp8,
                        'lhs1': st['lhs1'], 'lhs2': st['lhs2'],
                        'lhs8': st['lhs8'], 'box8': st['box8'],
                        'pats': st['pats'], 'gmask': st['gmask'],
                        'maskout': st['maskout'], 'rowsgn': st['rsg'][h]})
    res = run_bass_kernel_spmd(st['nc'], in_maps, core_ids=list(range(8)),
                               trace=trace, **kw)
    out = np.empty((4, 30, 768, 768), np.float32)
    for core in range(8):
        b, h = core // 2, core % 2
        out[b, :, h * 384:(h + 1) * 384, :] = \
            res.results[core]['out'].astype(np.float32)
    return out, res


def kernel(bayer):
    out, _ = _run(bayer, trace=False)
    return out


# revision 3
# speedup vs baseline: 1.0615x; 1.0180x over previous
"""Trainium2 Bass kernel for nn_BayerFeatureExtractor (v2: fp8 DoubleRow).

Input:  bayer [4, 1, 768, 768] f32.  Output: [4, 30, 768, 768] f32.

Sharding: data-parallel over 8 cores: core i handles batch b = i//2,
row-half h = i%2 (output rows [h*384, (h+1)*384)).

Per-core: 4 row-tiles (96 out rows) x 2 col-blocks (384 out cols).
All convolutions are banded matmuls contracting over image rows.
Precision-tolerant banks (k5 texture bank, gdm/gda/sumd/hxy/gcross/hf,
box5) run as fp8(e4m3) DoubleRow, two kernel-columns per pass at 0.5
cyc/col: the rhs is a [K, 2, N] view over a DUPLICATED fp8 tile whose
copies sit at an even gap so the k-tile j-stride is 4/16B aligned
(odd/unaligned strides and overlapping views crash the hw; validated
by probes).  All dx pairs are (dx, dx+2).  Precision-critical banks
(gx/gy, lap, hd, gh-gv, fills A-series, J) stay bf16.

Lane discipline: engines cannot shift partitions (start partition must
be 0/32/64/96), so ext-grid results ([100, 388], lane x = out row x-2)
cross to the central grid ([96, 384], lane = out row) only via DMA:
ext channels are packed into extG [100, 5, 388] / shiftE [100, 2, 388]
staging tiles and DMA'd (to DRAM directly, or to a central SBUF tile
for grad_mag/chroma/dir_var inputs).  The bayer identity tap needed by
gir is folded into the gcross kernel (gcross - delta).  aniso's +EPS on
(Jxx+Jyy) is a 1-partition eps-row matmul pass appended to the Jxx
series.  Central channels stage in stG [96, 21, 384] bf16 -> 3 HWDGE
DMAs per block; masks (ch 11-13) are one whole-core DMA.
"""
import math
import os
import sys
from contextlib import ExitStack

import numpy as np

for _p in ('/opt/trn_rl_repo', '/root/.axon_site/_ro/trn_rl_repo'):
    if os.path.isdir(_p) and _p not in sys.path:
        sys.path.insert(0, _p)

import concourse.bass as bass
import concourse.bacc as bacc
import concourse.mybir as mybir
import concourse.tile as tile
from concourse.ap import AP
from concourse.bass_utils import run_bass_kernel_spmd

F32 = mybir.dt.float32
BF16 = mybir.dt.bfloat16
FP8 = mybir.dt.float8e4
AL = mybir.AluOpType
AF = mybir.ActivationFunctionType
DRMODE = mybir.MatmulPerfMode.DoubleRow

EPS = 1e-6
K1, M1 = 104, 100    # ext contraction / rows
K2 = 100             # stage-2 contraction (= M1)
M2 = 96              # central rows

# ---------------------------------------------------------------------------
# constant kernels (identical math to reference._build_kernels)
# ---------------------------------------------------------------------------


def _norm(k):
    k = k - k.mean()
    return (k / max(float(np.abs(k).sum()), 1e-6)).astype(np.float32)


def _gabor(theta, size=5, sigma=1.1, wavelength=3.0, gamma=0.65):
    r = size // 2
    c = np.arange(-r, r + 1, dtype=np.float32)
    yy, xx = np.meshgrid(c, c, indexing='ij')
    xt = xx * math.cos(theta) + yy * math.sin(theta)
    yt = -xx * math.sin(theta) + yy * math.cos(theta)
    env = np.exp(-(xt ** 2 + (gamma * yt) ** 2) / (2.0 * sigma * sigma))
    return _norm(env * np.cos(2.0 * math.pi * xt / wavelength))


def _dct(size=5, u=2, v=2):
    c = np.arange(size, dtype=np.float32)
    return _norm(np.outer(np.cos(math.pi * (c + 0.5) * v / size),
                          np.cos(math.pi * (c + 0.5) * u / size)))


def build_kernels():
    f32 = np.float32
    k3 = np.stack([
        _norm(np.array([[-1, 0, 1], [-2, 0, 2], [-1, 0, 1]], f32)),
        _norm(np.array([[-1, -2, -1], [0, 0, 0], [1, 2, 1]], f32)),
        _norm(np.array([[-2, -1, 0], [-1, 0, 1], [0, 1, 2]], f32)),
        _norm(np.array([[0, 1, 2], [-1, 0, 1], [-2, -1, 0]], f32)),
        np.array([[0, 1, 0], [1, -4, 1], [0, 1, 0]], f32),
        np.array([[0, 0, 0], [1, -2, 1], [0, 0, 0]], f32),
        np.array([[0, 1, 0], [0, -2, 0], [0, 1, 0]], f32),
        np.array([[1, 0, -1], [0, 0, 0], [-1, 0, 1]], f32) / 4.0,
        np.array([[0, .25, 0], [.25, 0, .25], [0, .25, 0]], f32),
        _norm(np.array([[1, -2, 1], [-2, 4, -2], [1, -2, 1]], f32)),
    ])
    ii, jj = np.indices((5, 5))
    s = np.sin(2.0 * math.pi * np.arange(5, dtype=f32) / 5.0)
    c = np.cos(2.0 * math.pi * np.arange(5, dtype=f32) / 5.0)
    k5 = np.stack([
        _norm(((-1.0) ** (ii + jj)).astype(f32)),   # cb
        _norm(((-1.0) ** jj).astype(f32)),          # sh
        _norm(((-1.0) ** ii).astype(f32)),          # sv
        _norm(np.tile(s, (5, 1))),                  # sinx
        _norm(np.tile(s.reshape(5, 1), (1, 5))),    # siny
        _norm(np.tile(c, (5, 1))),                  # phx
        _norm(np.tile(c.reshape(5, 1), (1, 5))),    # phy
        _gabor(math.pi / 4.0),                      # g45
        _gabor(3.0 * math.pi / 4.0),                # g135
        _dct(),                                     # dct
    ])
    ha = np.array([-0.25, 0.5, 0.5, 0.5, -0.25], f32)
    t5 = np.array([1, 2, 3, 2, 1], f32) / 9.0
    return k3, k5, ha, t5


def pad5(col3):
    z = np.zeros(5, np.float32)
    z[1:4] = np.asarray(col3, np.float32)
    return z


def banded_ext(col5):
    B = np.zeros((K1, M1), np.float32)
    for x in range(M1):
        for dy in range(-2, 3):
            k = x + 2 + dy
            if 0 <= k < K1:
                B[k, x] = col5[dy + 2]
    return B


def banded_cen(col5):
    B = np.zeros((K1, M2), np.float32)
    for m in range(M2):
        for dy in range(-2, 3):
            k = m + 4 + dy
            if 0 <= k < K1:
                B[k, m] = col5[dy + 2]
    return B


def banded_s2(col5, scale=1.0):
    B = np.zeros((K2, M2), np.float32)
    for y in range(M2):
        for dy in range(-2, 3):
            k = y + 2 + dy
            if 0 <= k < K2:
                B[k, y] = col5[dy + 2] * scale
    return B


def fp8_quant(x):
    import ml_dtypes
    return np.asarray(x, np.float32).astype(
        ml_dtypes.float8_e4m3).astype(np.float32)


def best_ws(kern):
    k = np.asarray(kern, np.float32)
    best = None
    for e in range(-30, 31):
        s = 1.05 ** e
        err = float(np.abs(fp8_quant(k * s) / s - k).sum())
        if best is None or err < best[0]:
            best = (err, s)
    return best[1]


def cols_of(kern):
    k = np.asarray(kern, np.float32)
    if k.shape[0] == 3:
        kk = np.zeros((5, 5), np.float32)
        kk[1:4, 1:4] = k
        k = kk
    out = []
    for dx in range(-2, 3):
        col = k[:, dx + 2]
        if np.any(col != 0):
            out.append((dx, col.astype(np.float32)))
    return out


def make_pairs(dxs):
    """Pairs at distance exactly 2 (even j-stride gap required by hw);
    lone dx becomes a zero-padded pass, biased toward small dx0 so the
    padded k-tile window stays in-bounds."""
    rest = sorted(dxs, reverse=True)
    pairs = []
    while rest:
        x = rest.pop(0)
        if x - 2 in rest:
            rest.remove(x - 2)
            pairs.append((x - 2, x))
        else:
            pairs.append((x, None))
    return list(reversed(pairs))


# ---------------------------------------------------------------------------
# weight packs
# ---------------------------------------------------------------------------

def build_packs():
    k3, k5, ha, t5 = build_kernels()

    mats = []
    bidx = {}

    def addb(name, mlist, dxs=None):
        bidx[name] = (len(mats), len(mlist))
        if dxs is not None:
            bidx[name + '_dx'] = dxs
        mats.extend(mlist)

    for nm, kern in (('gx', k3[0]), ('gy', k3[1])):
        coll = cols_of(kern)
        addb(nm, [banded_ext(c) for dx, c in coll], [dx for dx, c in coll])
    for nm, kern in (('lap', k3[4]), ('hd', k3[5] - k3[6])):
        coll = cols_of(kern)
        addb(nm, [banded_cen(c) for dx, c in coll], [dx for dx, c in coll])
    I5 = pad5([0, 1, 0])
    addb('ghgv0', [banded_cen(I5) * ha[dx + 2] for dx in (-2, -1, 1, 2)],
         [-2, -1, 1, 2])
    addb('ghgvC', [banded_cen(I5) * ha[2] - banded_cen(ha)])
    gcmi = k3[8].copy()
    gcmi[1, 1] -= 1.0   # gcross - delta: psum = gcross*b - b
    coll = cols_of(gcmi)
    addb('gcmi', [banded_cen(c) for dx, c in coll], [dx for dx, c in coll])
    kk = (np.arange(K1) % 2).astype(np.float32)
    Bod = banded_ext(t5) * kk[:, None]
    Bev = banded_ext(t5) * (1.0 - kk)[:, None]
    fm = []
    fa_start = len(mats)
    for aname, Ba in (('O', Bod), ('E', Bev)):
        for dx in range(-2, 3):
            grp = ('Ae' if dx % 2 == 0 else 'Ao') + aname
            fm.append((grp, dx))
            mats.append(Ba * t5[dx + 2])
    bidx['fillsA'] = (fa_start, 10)
    bidx['fillsA_meta'] = fm
    lhs1 = np.zeros((K1, len(mats) * M1), np.float32)
    for i, m in enumerate(mats):
        lhs1[:, i * M1:i * M1 + m.shape[1]] = m

    # stage-2 bf16: J series [K2, 5, 96]
    lhs2 = np.concatenate(
        [banded_s2(t5, t5[dx + 2]) for dx in range(-2, 3)], axis=1)

    # fp8 stage-1 DoubleRow
    f8kern = {'gdm': k3[2], 'gda': k3[3],
              'sumd': k3[0] + k3[1] + k3[2] + k3[3],
              'hxy': k3[7], 'hf': k3[9],
              'cb': k5[0], 'sh': k5[1], 'sv': k5[2], 'sinx': k5[3],
              'siny': k5[4], 'phx': k5[5], 'phy': k5[6], 'g45': k5[7],
              'g135': k5[8], 'dct': k5[9]}
    f8packs = []
    f8idx = {}
    for nm, kern in f8kern.items():
        ws = 1.0 if nm in ('sinx', 'siny') else best_ws(kern)
        coll = cols_of(kern)
        cold = {dx: c for dx, c in coll}
        pl = []
        start = len(f8packs)
        for dx0, dx1 in make_pairs([dx for dx, c in coll]):
            W = np.zeros((K1, 2, M2), np.float32)
            W[:, 0, :] = banded_cen(cold[dx0]) * ws
            if dx1 is not None:
                W[:, 1, :] = banded_cen(cold[dx1]) * ws
            f8packs.append(W)
            pl.append(dx0)
        f8idx[nm] = (start, pl, ws)
    lhs8 = np.zeros((K1, len(f8packs) * 2 * M2), np.float32)
    for i, W in enumerate(f8packs):
        lhs8[:, i * 2 * M2:(i + 1) * 2 * M2] = W.reshape(K1, 2 * M2)

    # fp8 stage-2 box: taps 0.25*0.25 exact; true scale 16/25
    BOX_SCALE = 16.0 / 25.0
    ones5 = np.ones(5, np.float32)
    box_pairs = make_pairs(range(-2, 3))
    box8 = np.zeros((K2, len(box_pairs) * 2 * M2), np.float32)
    box_meta = []
    for i, (dx0, dx1) in enumerate(box_pairs):
        W = np.zeros((K2, 2, M2), np.float32)
        W[:, 0, :] = banded_s2(ones5 * 0.25, 0.25)
        if dx1 is not None:
            W[:, 1, :] = banded_s2(ones5 * 0.25, 0.25)
        box8[:, i * 2 * M2:(i + 1) * 2 * M2] = W.reshape(K2, 2 * M2)
        box_meta.append(dx0)

    return dict(lhs1=lhs1, bidx=bidx, lhs2=lhs2, lhs8=lhs8, f8idx=f8idx,
                n8=len(f8packs), box8=box8, box_meta=box_meta,
                BOX_SCALE=BOX_SCALE, nb=len(mats))


def build_patterns():
    t5 = np.array([1, 2, 3, 2, 1], np.float32) / 9.0

    def mfun(ch, rp, cp):
        return {
            'r': float(rp == 1 and cp == 0),
            'b': float(rp == 0 and cp == 1),
            'gr': float(rp == 1 and cp == 1),
            'gb': float(rp == 0 and cp == 0),
            'g': float((rp == 1 and cp == 1) or (rp == 0 and cp == 0)),
        }[ch]

    P, W = 128, 776
    pp = np.arange(P)[:, None] % 2
    cc = np.arange(W)[None, :] % 2
    pats = []
    for ch in ['r', 'b', 'g', 'gr', 'gb']:
        v = np.zeros((2, 2), np.float32)
        for rp in range(2):
            for cp in range(2):
                d = sum(t5[dy + 2] * t5[dx + 2]
                        * mfun(ch, (rp + dy) % 2, (cp + dx) % 2)
                        for dy in range(-2, 3) for dx in range(-2, 3))
                v[rp, cp] = 1.0 / max(d, EPS)
        pats.append(v[pp, cc].astype(np.float32))
    gmask = np.array([[mfun('g', rp, cp) for cp in range(2)]
                      for rp in range(2)], np.float32)[pp, cc]
    return np.stack(pats), gmask.astype(np.float32)


def build_maskout():
    er = (np.arange(384) % 2 == 0).astype(np.float32)[:, None]
    ec = (np.arange(768) % 2 == 0).astype(np.float32)[None, :]
    gb_m = er * ec
    b_m = er * (1.0 - ec)
    r_m = (1.0 - er) * ec
    gr_m = (1.0 - er) * (1.0 - ec)
    g_m = gr_m + gb_m
    return np.stack([r_m, g_m, b_m])


def build_rowsign(h):
    sg = np.ones((4, M1), np.float32)
    for t in range(4):
        for x in range(M1):
            r = h * 384 + 96 * t - 2 + x
            if r < 0 or r >= 768:
                sg[t, x] = -1.0
    return sg.T.copy()  # [100, 4]


# central staging slots: 0..8 = ch2..10; 9..10 = ch17..18; 11..20 = ch20..29;
# 21 = ch19 (chroma, written by its own DMA)
CSLOT = {'gdm': 0, 'gda': 1, 'grad_mag': 2, 'lap': 3, 'lam_max': 4,
         'lam_min': 5, 'aniso': 6, 'dir_var': 7, 'orient_e': 8,
         'gir': 9, 'dgc': 10, 'cdv': 11, 'cb_e': 12, 'sh': 13, 'sv': 14,
         'ax': 15, 'ay': 16, 'phase_e': 17, 'sx': 18, 'sy': 19,
         'highband': 20, 'chroma': 21}
# ext staging slots (extG): gx, gy -> ch0,1; gpd, rg, bg -> ch14,15,16
ESLOT = {'gx': 0, 'gy': 1, 'gpd': 2, 'rg': 3, 'bg': 4, 'chroma_sq': 5}


# ---------------------------------------------------------------------------
# kernel builder
# ---------------------------------------------------------------------------

def build_nc():
    packs = build_packs()
    bidx = packs['bidx']
    f8idx = packs['f8idx']
    BOXS = packs['BOX_SCALE']
    NB = packs['nb']
    N8 = packs['n8']
    NBX = len(packs['box_meta'])

    nc = bacc.Bacc(None, target_bir_lowering=False)
    bay_d = nc.dram_tensor('bayer_pad', [392, 776], BF16, kind='ExternalInput')
    bay8_d = nc.dram_tensor('bayer_pad8', [392, 776], FP8,
                            kind='ExternalInput')
    lhs1_d = nc.dram_tensor('lhs1', [K1, NB * M1], BF16, kind='ExternalInput')
    lhs2_d = nc.dram_tensor('lhs2', [K2, 5 * M2], BF16, kind='ExternalInput')
    lhs8_d = nc.dram_tensor('lhs8', [K1, N8 * 2 * M2], FP8,
                            kind='ExternalInput')
    box8_d = nc.dram_tensor('box8', [K2, NBX * 2 * M2], FP8,
                            kind='ExternalInput')
    pat_d = nc.dram_tensor('pats', [128, 5 * 2], BF16, kind='ExternalInput')
    gm_d = nc.dram_tensor('gmask', [128, 776], BF16, kind='ExternalInput')
    rsg_d = nc.dram_tensor('rowsgn', [M1, 4], F32, kind='ExternalInput')
    mo_d = nc.dram_tensor('maskout', [3, 384, 768], BF16,
                          kind='ExternalInput')
    out_d = nc.dram_tensor('out', [30, 384, 768], BF16, kind='ExternalOutput')

    with tile.TileContext(nc) as tc, ExitStack() as ctx:
        cpool = ctx.enter_context(tc.tile_pool(name='const', bufs=1))
        inpool = ctx.enter_context(tc.tile_pool(name='inp', bufs=2))
        wpool = ctx.enter_context(tc.tile_pool(name='work', bufs=2))
        spool = ctx.enter_context(tc.tile_pool(name='stage', bufs=2))
        pspool = ctx.enter_context(
            tc.tile_pool(name='ps', bufs=8, space='PSUM'))

        epsT = cpool.tile([128, 1], F32, tag='epsT', name='epsT')
        eps4T = cpool.tile([128, 1], F32, tag='eps4T', name='eps4T')
        onesW = cpool.tile([1, M2], BF16, tag='onesW', name='onesW')
        epsRow = cpool.tile([1, 384], BF16, tag='epsRow', name='epsRow')
        nc.vector.memset(epsT[:], EPS)
        nc.vector.memset(eps4T[:], 4.0 * EPS)
        nc.vector.memset(onesW[:], 1.0)
        nc.vector.memset(epsRow[:], EPS)
        zeroT = cpool.tile([128, 384], BF16, tag='zeroT', name='zeroT')
        nc.vector.memset(zeroT[:], 0.0)
        lhs1_t = cpool.tile([K1, NB * M1], BF16, tag='lhs1')
        lhs2_t = cpool.tile([K2, 5, M2], BF16, tag='lhs2')
        lhs8_t = cpool.tile([K1, N8, 2, M2], FP8, tag='lhs8')
        box8_t = cpool.tile([K2, NBX, 2, M2], FP8, tag='box8')
        pat_t = cpool.tile([128, 5, 2], BF16, tag='pats')
        gm_t = cpool.tile([128, 776], BF16, tag='gmask')
        rsg_t = cpool.tile([M1, 4], F32, tag='rsg')
        nc.sync.dma_start(lhs1_t[:], lhs1_d[:])
        nc.sync.dma_start(
            lhs8_t[:], lhs8_d[:].rearrange('k (n j m) -> k n j m',
                                           n=N8, j=2))
        nc.sync.dma_start(
            pat_t[:], pat_d[:].rearrange('p (n w) -> p n w', n=5))  # [128,5,2]
        nc.sync.dma_start(rsg_t[:], rsg_d[:])
        nc.sync.dma_start(gm_t[:], gm_d[:])
        nc.sync.dma_start(
            lhs2_t[:], lhs2_d[:].rearrange('k (n m) -> k n m', n=5))
        nc.sync.dma_start(
            box8_t[:], box8_d[:].rearrange('k (n j m) -> k n j m',
                                           n=NBX, j=2))
        ctxd = dict(nc=nc, packs=packs, bidx=bidx, f8idx=f8idx, BOXS=BOXS,
                    zeroT=zeroT,
                    lhs1_t=lhs1_t, lhs2_t=lhs2_t, lhs8_t=lhs8_t,
                    box8_t=box8_t, pat_t=pat_t, gm_t=gm_t, rsg_t=rsg_t,
                    epsT=epsT, eps4T=eps4T, onesW=onesW, epsRow=epsRow,
                    out_d=out_d, wpool=wpool, spool=spool, pspool=pspool)

        pending = None
        for t in range(4):
            r0 = 96 * t
            BT = inpool.tile([K1, 776], BF16, tag='BT')
            BT8 = inpool.tile([K1, 2, 782], FP8, tag='BT8')
            nc.sync.dma_start(BT[:], bay_d[r0:r0 + 104, :])
            nc.sync.dma_start(BT8[:, 0, 0:776], bay8_d[r0:r0 + 104, :])
            nc.sync.dma_start(BT8[:, 1, 0:776], bay8_d[r0:r0 + 104, :])
            for cbi in range(2):
                stA = build_block_A(ctxd, t, cbi, BT, BT8)
                if pending is not None:
                    build_block_B(ctxd, pending)
                pending = stA
        build_block_B(ctxd, pending)
        nc.sync.dma_start(out_d[11:14, :, :], mo_d[:])

    nc.compile()
    return nc, packs


def _helpers(C, t, cbi):
    nc = C['nc']
    lhs1_t = C['lhs1_t']
    bidx = C['bidx']
    epsT, eps4T = C['epsT'], C['eps4T']
    wpool, pspool = C['wpool'], C['pspool']

    def l1(name, i=0, M=M2):
        s, _ = bidx[name]
        return lhs1_t[:, (s + i) * M1:(s + i) * M1 + M]

    def ps_new(shape, tag):
        return pspool.tile(list(shape), F32, tag='ps', name=tag)

    def wt(tag, shape=(M2, 384), dt=BF16):
        return wpool.tile(list(shape), dt, tag=tag, name=tag)

    def act(out, in_, func, bias=0.0, scale=1.0):
        if isinstance(bias, float) and bias != 0.0:
            bt = eps4T if bias == 4.0 * EPS else epsT
            bias = bt[0:out.shape[0], :]
        nc.scalar.activation(out, in_, func, bias=bias, scale=scale)

    def MM(ps, lh, rh, start, stop):
        nc.tensor.matmul(ps, lh, rh, start=start, stop=stop)

    def drview(dup_tile, pre, coff, npart, N, W):
        sl = dup_tile[pre + (slice(coff, coff + N),)]
        return AP(sl.tensor, sl.offset,
                  [list(sl.ap[0]), [W + 2, 2], [1, N]])

    return l1, ps_new, wt, act, MM, drview


def build_block_A(C, t, cbi, BT, BT8):
    """Stage A: all stage-1 matmul series + fills/product vector work.
    Returns state consumed by build_block_B one block later."""
    nc = C['nc']
    packs, bidx, f8idx = C['packs'], C['bidx'], C['f8idx']
    lhs8_t = C['lhs8_t']
    pat_t, gm_t, rsg_t = C['pat_t'], C['gm_t'], C['rsg_t']
    out_d = C['out_d']
    spool = C['spool']
    V = nc.vector
    P = nc.gpsimd
    r0 = 96 * t
    c0 = 384 * cbi
    ev = np.s_[:, 0::2]
    od = np.s_[:, 1::2]
    CENF = np.s_[2:386]
    l1, ps_new, wt, act, MM, drview = _helpers(C, t, cbi)

    def bserE(name, tag):
        p = ps_new([M1, 388], tag)
        dxs = bidx[name + '_dx']
        for i, dx in enumerate(dxs):
            MM(p[:], l1(name, i, M1),
               BT[:, c0 + 2 + dx:c0 + 2 + dx + 388],
               i == 0, i == len(dxs) - 1)
        return p

    def bserC(name, tag, stop=True):
        p = ps_new([M2, 384], tag)
        dxs = bidx[name + '_dx']
        for i, dx in enumerate(dxs):
            MM(p[:], l1(name, i), BT[:, c0 + 4 + dx:c0 + 4 + dx + 384],
               i == 0, stop and i == len(dxs) - 1)
        return p

    def f8ser(name, tag):
        p = ps_new([M2, 384], tag)
        start, pl, ws = f8idx[name]
        for i, dx0 in enumerate(pl):
            rhs = drview(BT8, (slice(0, K1), 0), c0 + 4 + dx0, K1, 384, 782)
            nc.tensor.matmul(p[:], lhs8_t[:, start + i, :, :], rhs,
                             start=(i == 0), stop=(i == len(pl) - 1),
                             perf_mode=DRMODE)
        return p

    stG = spool.tile([M2, 22, 384], BF16, tag='stG')
    extG = spool.tile([M1, 6, 388], BF16, tag='extG')
    shiftC = spool.tile([M2, 2, 384], BF16, tag='shiftC')

    def stg(name):
        return stG[:, CSLOT[name], :]

    def ext(name):
        return extG[:, ESLOT[name], :]

    # ===== PE: ext gradients + fills A-series =====
    gxP = bserE('gx', 'gxP')
    gyP = bserE('gy', 'gyP')
    fa_start, _ = bidx['fillsA']
    fa_meta = bidx['fillsA_meta']
    Aps = {}
    for g in ('AeO', 'AoO', 'AeE', 'AoE'):
        idxs = [i for i, (gg, dx) in enumerate(fa_meta) if gg == g]
        p = ps_new([M1, 388], 'A' + g)
        for j, i in enumerate(idxs):
            dx = fa_meta[i][1]
            MM(p[:], l1('fillsA', i, M1),
               BT[:, c0 + 2 + dx:c0 + 2 + dx + 388],
               j == 0, j == len(idxs) - 1)
        Aps[g] = p

    act(ext('gx'), gxP[:], AF.Copy)
    act(ext('gy'), gyP[:], AF.Copy)
    As = {g: wt('As' + g, (M1, 388)) for g in ('AeE', 'AoE')}
    act(As['AeE'][:], Aps['AeE'][:], AF.Copy)
    act(As['AoE'][:], Aps['AoE'][:], AF.Copy)

    # gradient products (feed J in stage B)
    gx2 = wt('gx2', (M1, 388))
    gy2 = wt('gy2', (M1, 388))
    gxyF = wt('gxyF', (M1, 388))
    V.tensor_mul(gx2[:], ext('gx'), ext('gx'))
    V.tensor_mul(gy2[:], ext('gy'), ext('gy'))
    rsg = rsg_t[0:M1, t:t + 1]
    V.scalar_tensor_tensor(gxyF[:], ext('gx'), rsg, ext('gy'),
                           AL.mult, AL.mult)
    if cbi == 0:
        V.tensor_scalar(gxyF[:, 0:2], gxyF[:, 0:2], -1.0, None, AL.mult)
    else:
        V.tensor_scalar(gxyF[:, 386:388], gxyF[:, 386:388], -1.0, None,
                        AL.mult)
    ssqE = wt('ssqE', (M1, 388))
    V.tensor_add(ssqE[:], gx2[:], gy2[:])
    d2E = wt('d2E', (M1, 388))
    V.tensor_sub(d2E[:], gx2[:], gy2[:])

    # ===== DVE: fills chain =====
    def pv(pi, par):
        # master col (c0+2+j) parity == j parity (c0 even): broadcast the
        # per-(row-parity, col-parity) scalar across the 194 half columns
        return pat_t[0:M1, pi, par:par + 1].to_broadcast([M1, 194])

    IVRe, IVRo = pv(0, 0), pv(0, 1)
    IVBe, IVBo = pv(1, 0), pv(1, 1)
    IVGe, IVGo = pv(2, 0), pv(2, 1)
    IVGRe, IVGRo = pv(3, 0), pv(3, 1)
    IVGBe, IVGBo = pv(4, 0), pv(4, 1)
    fr = wt('fr', (M1, 388))
    fb = wt('fb', (M1, 388))
    tg = wt('tg', (M1, 388))
    fg = wt('fg', (M1, 388))
    fgr = wt('fgr', (M1, 388))
    fgb = wt('fgb', (M1, 388))
    V.tensor_mul(fr[ev], Aps['AeO'][ev], IVRe)
    V.tensor_mul(fr[od], Aps['AoO'][od], IVRo)
    V.tensor_mul(fb[ev], As['AoE'][ev], IVBe)
    V.tensor_mul(fb[od], As['AeE'][od], IVBo)
    V.tensor_add(tg[ev], Aps['AoO'][ev], As['AeE'][ev])
    V.tensor_add(tg[od], Aps['AeO'][od], As['AoE'][od])
    P.tensor_mul(fg[ev], tg[ev], IVGe)
    P.tensor_mul(fg[od], tg[od], IVGo)
    V.tensor_mul(fgr[ev], Aps['AoO'][ev], IVGRe)
    V.tensor_mul(fgr[od], Aps['AeO'][od], IVGRo)
    V.tensor_mul(fgb[ev], As['AeE'][ev], IVGBe)
    V.tensor_mul(fgb[od], As['AoE'][od], IVGBo)
    V.tensor_sub(ext('gpd'), fgr[:], fgb[:])
    V.tensor_sub(ext('rg'), fr[:], fg[:])
    V.tensor_sub(ext('bg'), fb[:], fg[:])

    dup4 = wt('dup4', (M1, 4, 2, 390), FP8)
    P.tensor_copy(dup4[:, 0, 0, 0:388], ext('rg'))
    P.tensor_copy(dup4[:, 1, 0, 0:388], ext('bg'))
    rgsq = wt('rgsq', (M1, 388))
    V.tensor_mul(rgsq[:], ext('rg'), ext('rg'))
    bgsq = wt('bgsq', (M1, 388))
    V.tensor_mul(bgsq[:], ext('bg'), ext('bg'))

    shiftE = wt('shiftE', (M1, 2, 388))
    V.tensor_copy(shiftE[:, 0, :], ssqE[:])
    V.tensor_add(shiftE[:, 1, :], rgsq[:], bgsq[:])
    nc.sync.dma_start(shiftC[:], shiftE[2:98, :, CENF])
    P.tensor_copy(dup4[:, 2, 0, 0:388], rgsq[:])
    P.tensor_copy(dup4[:, 3, 0, 0:388], bgsq[:])
    nc.sync.dma_start(dup4[:, :, 1, 0:388], dup4[:, :, 0, 0:388])

    # ===== PE: fp8 + bf16 central series, drains interleaved =====
    gdmP = f8ser('gdm', 'gdmP')
    V.tensor_copy(stg('gdm'), gdmP[:])
    gdaP = f8ser('gda', 'gdaP')
    V.tensor_copy(stg('gda'), gdaP[:])
    _, _, ws_smd = f8idx['sumd']
    smdP = f8ser('sumd', 'smdP')
    msq = wt('msq')
    act(msq[:], smdP[:], AF.Square, scale=0.25 / ws_smd)
    sq1 = wt('sq1')
    V.tensor_mul(sq1[:], stg('gdm'), stg('gdm'))
    sq2 = wt('sq2')
    V.tensor_mul(sq2[:], stg('gda'), stg('gda'))

    lapP = bserC('lap', 'lapP')
    V.tensor_copy(stg('lap'), lapP[:])
    hdP = bserC('hd', 'hdP')
    _, _, ws_hxy = f8idx['hxy']
    hxyP = f8ser('hxy', 'hxyP')
    hd2 = wt('hd2')
    act(hd2[:], hdP[:], AF.Square, scale=0.5)
    hxy2 = wt('hxy2')
    act(hxy2[:], hxyP[:], AF.Square, scale=1.0 / ws_hxy)
    hq = wt('hq')
    P.tensor_add(hq[:], hd2[:], hxy2[:])
    hs = wt('hs')
    act(hs[:], hq[:], AF.Sqrt, bias=EPS)
    V.scalar_tensor_tensor(stg('lam_max'), lapP[:], 0.5, hs[:], AL.mult,
                           AL.add)
    V.scalar_tensor_tensor(stg('lam_min'), lapP[:], 0.5, hs[:], AL.mult,
                           AL.subtract)

    gcrP = bserC('gcmi', 'gcrP')
    V.scalar_tensor_tensor(stg('gir'), gcrP[:], -1.0,
                           gm_t[0:M2, c0 + 4:c0 + 388], AL.mult, AL.mult)

    ghvP = bserC('ghgv0', 'ghvP', stop=False)
    MM(ghvP[:], l1('ghgvC'), BT[:, c0 + 4:c0 + 4 + 384], False, True)
    act(stg('dgc'), ghvP[:], AF.Abs)

    def sq_pair(nameA, nameB, outname, tagp):
        _, _, wsA = f8idx[nameA]
        _, _, wsB = f8idx[nameB]
        pA = f8ser(nameA, tagp + 'A')
        pB = f8ser(nameB, tagp + 'B')
        a2 = wt(tagp + 'a2')
        act(a2[:], pA[:], AF.Square, scale=1.0 / wsA)
        b2 = wt(tagp + 'b2')
        act(b2[:], pB[:], AF.Square, scale=1.0 / wsB)
        q = wt(tagp + 'q')
        P.tensor_add(q[:], a2[:], b2[:])
        act(stg(outname), q[:], AF.Sqrt, bias=EPS)

    sq_pair('hf', 'dct', 'highband', 'hb')
    sq_pair('phx', 'phy', 'phase_e', 'ph')
    sq_pair('g45', 'g135', 'orient_e', 'oe')

    _, _, ws_cb = f8idx['cb']
    cbP = f8ser('cb', 'cbP')
    act(stg('cb_e'), cbP[:], AF.Abs, scale=1.0 / ws_cb)

    for nm, slot in (('sh', 'sh'), ('sv', 'sv')):
        _, _, ws = f8idx[nm]
        p = f8ser(nm, 'p' + nm)
        act(stg(slot), p[:], AF.Copy, scale=1.0 / ws)
    for nm, slot in (('sinx', 'sx'), ('siny', 'sy')):
        p = f8ser(nm, 'p' + nm)
        V.tensor_copy(stg(slot), p[:])

    # ext channels straight to DRAM
    nc.sync.dma_start(
        out_d[0:2, r0:r0 + 96, c0:c0 + 384].rearrange('n p w -> p n w'),
        extG[2:98, 0:2, CENF])
    nc.sync.dma_start(
        out_d[14:17, r0:r0 + 96, c0:c0 + 384].rearrange('n p w -> p n w'),
        extG[2:98, 2:5, CENF])

    return dict(t=t, cbi=cbi, stG=stG, shiftC=shiftC, dup4=dup4,
                gxyF=gxyF, ssqE=ssqE, d2E=d2E, msq=msq, sq1=sq1, sq2=sq2)


def build_block_B(C, S):
    """Stage B: J + box matmuls and tails for the PREVIOUS block."""
    nc = C['nc']
    packs, f8idx, BOXS = C['packs'], C['f8idx'], C['BOXS']
    lhs2_t, box8_t = C['lhs2_t'], C['box8_t']
    onesW, epsRow = C['onesW'], C['epsRow']
    out_d = C['out_d']
    V = nc.vector
    P = nc.gpsimd
    t, cbi = S['t'], S['cbi']
    r0 = 96 * t
    c0 = 384 * cbi
    l1, ps_new, wt, act, MM, drview = _helpers(C, t, cbi)
    stG, shiftC, dup4 = S['stG'], S['shiftC'], S['dup4']
    gxyF, ssqE, d2E = S['gxyF'], S['ssqE'], S['d2E']
    msq, sq1, sq2 = S['msq'], S['sq1'], S['sq2']

    def stg(name):
        return stG[:, CSLOT[name], :]

    # ===== PE: smooth5 of (gx2-gy2), (gx2+gy2)+eps, gxy =====
    djP = ps_new([M2, 384], 'djP')
    smP = ps_new([M2, 384], 'smP')
    jxyP = ps_new([M2, 384], 'jxyP')
    for i, dx in enumerate(range(-2, 3)):
        MM(djP[:], lhs2_t[:, i, :], d2E[:, 2 + dx:2 + dx + 384],
           i == 0, i == 4)
        MM(jxyP[:], lhs2_t[:, i, :], gxyF[:, 2 + dx:2 + dx + 384],
           i == 0, i == 4)
        MM(smP[:], lhs2_t[:, i, :], ssqE[:, 2 + dx:2 + dx + 384],
           i == 0, False)
    nc.tensor.matmul(smP[:], onesW[:], epsRow[:], start=False, stop=True)

    dj2 = wt('dj2')
    act(dj2[:], djP[:], AF.Square, scale=0.5)
    jxy2 = wt('jxy2')
    act(jxy2[:], jxyP[:], AF.Square)
    qj = wt('qj')
    P.tensor_add(qj[:], dj2[:], jxy2[:])
    anum = wt('anum')
    act(anum[:], qj[:], AF.Sqrt, bias=4.0 * EPS, scale=4.0)
    rec = wt('rec', dt=F32)
    V.reciprocal(rec[:], smP[:])
    P.tensor_mul(stg('aniso'), anum[:], rec[:])

    # ===== PE: box (fp8 DR stage-2) =====
    def boxser(slot, tag):
        p = ps_new([M2, 384], tag)
        for i, dx0 in enumerate(packs['box_meta']):
            rhs = drview(dup4, (slice(0, K2), slot, 0), 2 + dx0,
                         K2, 384, 390)
            nc.tensor.matmul(p[:], box8_t[:, i, :, :], rhs,
                             start=(i == 0),
                             stop=(i == len(packs['box_meta']) - 1),
                             perf_mode=DRMODE)
        return p

    m1r = boxser(0, 'm1r')
    m1b = boxser(1, 'm1b')
    m2r = boxser(2, 'm2r')
    m2b = boxser(3, 'm2b')
    q1 = wt('q1')
    act(q1[:], m1r[:], AF.Square, scale=BOXS)
    q2 = wt('q2')
    act(q2[:], m1b[:], AF.Square, scale=BOXS)
    v1 = wt('v1')
    V.scalar_tensor_tensor(v1[:], m2r[:], BOXS, q1[:], AL.mult, AL.subtract)
    v2 = wt('v2')
    V.scalar_tensor_tensor(v2[:], m2b[:], BOXS, q2[:], AL.mult, AL.subtract)
    v1m = wt('v1m')
    V.tensor_scalar(v1m[:], v1[:], 0.0, None, AL.max)
    v2m = wt('v2m')
    V.tensor_scalar(v2m[:], v2[:], 0.0, None, AL.max)
    P.tensor_add(stg('cdv'), v1m[:], v2m[:])

    # ===== shiftC-dependent tail =====
    act(stg('grad_mag'), shiftC[:, 0, :], AF.Sqrt, bias=EPS)
    act(stg('chroma'), shiftC[:, 1, :], AF.Sqrt, bias=EPS)
    qa = wt('qa')
    P.tensor_add(qa[:], shiftC[:, 0, :], sq1[:])
    qb = wt('qb')
    P.tensor_add(qb[:], qa[:], sq2[:])
    V.scalar_tensor_tensor(stg('dir_var'), qb[:], 0.25, msq[:], AL.mult,
                           AL.subtract)

    # ===== stG output DMAs =====
    nc.sync.dma_start(
        out_d[2:11, r0:r0 + 96, c0:c0 + 384].rearrange('n p w -> p n w'),
        stG[:, 0:9, :])
    nc.sync.dma_start(
        out_d[17:19, r0:r0 + 96, c0:c0 + 384].rearrange('n p w -> p n w'),
        stG[:, 9:11, :])
    nc.sync.dma_start(
        out_d[20:24, r0:r0 + 96, c0:c0 + 384].rearrange('n p w -> p n w'),
        stG[:, 11:15, :])
    nc.sync.dma_start(
        out_d[26:30, r0:r0 + 96, c0:c0 + 384].rearrange('n p w -> p n w'),
        stG[:, 17:21, :])
    nc.sync.dma_start(out_d[19, r0:r0 + 96, c0:c0 + 384],
                      stG[:, CSLOT['chroma'], :])
    nc.sync.dma_start(
        out_d[24:26, r0:r0 + 96, c0:c0 + 384].rearrange('n p w -> p n w'),
        stG[:, CSLOT['sh']:CSLOT['sv'] + 1, :])


# ---------------------------------------------------------------------------
# host wrapper
# ---------------------------------------------------------------------------

_STATE = {}


def _get_state():
    if 'nc' not in _STATE:
        import ml_dtypes
        nc, packs = build_nc()
        pats, gmask = build_patterns()
        _STATE.update(
            nc=nc,
            lhs1=np.ascontiguousarray(
                packs['lhs1'].astype(ml_dtypes.bfloat16)),
            lhs2=np.ascontiguousarray(
                packs['lhs2'].astype(ml_dtypes.bfloat16)),
            lhs8=np.ascontiguousarray(
                packs['lhs8'].astype(ml_dtypes.float8_e4m3)),
            box8=np.ascontiguousarray(
                packs['box8'].astype(ml_dtypes.float8_e4m3)),
            pats=np.ascontiguousarray(
                pats[:, :, 0:2].transpose(1, 0, 2).reshape(128, 10)
                .astype(ml_dtypes.bfloat16)),
            gmask=np.ascontiguousarray(gmask.astype(ml_dtypes.bfloat16)),
            maskout=np.ascontiguousarray(
                build_maskout().astype(ml_dtypes.bfloat16)),
            rsg=[np.ascontiguousarray(build_rowsign(0)),
                 np.ascontiguousarray(build_rowsign(1))])
    return _STATE


def _run(bayer, trace=False, **kw):
    import ml_dtypes
    st = _get_state()
    bayer = np.ascontiguousarray(np.asarray(bayer, dtype=np.float32))
    in_maps = []
    for core in range(8):
        b, h = core // 2, core % 2
        Pimg = np.pad(bayer[b, 0], 4, mode='reflect')
        sl = Pimg[h * 384:h * 384 + 392, :]
        bp = np.ascontiguousarray(sl.astype(ml_dtypes.bfloat16))
        bp8 = np.ascontiguousarray(sl.astype(ml_dtypes.float8_e4m3))
        in_maps.append({'bayer_pad': bp, 'bayer_pad8': bp8,
                        'lhs1': st['lhs1'], 'lhs2': st['lhs2'],
                        'lhs8': st['lhs8'], 'box8': st['box8'],
                        'pats': st['pats'], 'gmask': st['gmask'],
                        'maskout': st['maskout'], 'rowsgn': st['rsg'][h]})
    res = run_bass_kernel_spmd(st['nc'], in_maps, core_ids=list(range(8)),
                               trace=trace, **kw)
    out = np.empty((4, 30, 768, 768), np.float32)
    for core in range(8):
        b, h = core // 2, core % 2
        out[b, :, h * 384:(h + 1) * 384, :] = \
            res.results[core]['out'].astype(np.float32)
    return out, res


def kernel(bayer):
    out, _ = _run(bayer, trace=False)
    return out
